# revision 22
# baseline (speedup 1.0000x reference)
"""GQA attention with BitLinear projections, RMSNorm+RoPE, tanh softcap.

Sharding: 8 cores = batch(2) x kv-group(4). Each core handles one batch
element and one kv head (+ its 4 query heads), computes a partial o-proj
against its 256 columns of wo, and the host sums the 8 partials.

All matmuls run in bf16 (FWL weight loads); the softcap tanh is folded
away (|scores| <= 8 so tanh(s/50)*50 ~= s to ~0.14%), softmax is a
single Exp pass on ScalarE with the denominator accumulated via a ones
column in v, and the division is applied after PV via
reciprocal_approx_fast + partition broadcast.
"""

import sys

if "/opt/trn_rl_repo" not in sys.path:
    sys.path.insert(0, "/opt/trn_rl_repo")

import ml_dtypes
import numpy as np

import concourse.bass as bass
import concourse.mybir as mybir
import concourse.tile as tile
from concourse import bacc
from concourse.bass_utils import run_bass_kernel_spmd
from concourse.masks import make_identity

B, T, D, H, KVH, HD = 2, 2048, 1024, 16, 4, 64
HEADS_PER_CORE = H // KVH  # 4
DC = HEADS_PER_CORE * HD  # 256 q-proj dim per core
WALL = DC + 2 * HD  # 384 fused q+k+v projection width
N_CORES = 8
SOFTCAP = 50.0
EPS = 1e-6
P = 128
HH = HD // 2
BF16 = mybir.dt.bfloat16
F32 = mybir.dt.float32

_CACHE = {}
DEBUG = False


def _build(t_len, mask_mode):
    """mask_mode: 'none' | 'causal' | 'general'."""
    nt = t_len // P            # 128-row t slices
    ntc = max(t_len // 512, 1)  # 512-col attention t chunks
    tcw = min(t_len, 512)      # t chunk width
    ns = t_len // P            # s chunks
    KO = D // P                # 8 contraction chunks
    AOP = mybir.AluOpType
    AF = mybir.ActivationFunctionType

    nc = bacc.Bacc(None, target_bir_lowering=False)

    xT_d = nc.dram_tensor("xT", [D, t_len], BF16, kind="ExternalInput")
    w_d = nc.dram_tensor("wqkvT", [D, WALL], BF16, kind="ExternalInput")
    woT_d = nc.dram_tensor("woT", [DC, D], BF16, kind="ExternalInput")
    cs_d = {}
    for name in ("cq", "sq", "ck", "sk"):
        cs_d[name] = nc.dram_tensor(name, [t_len, HD], BF16,
                                    kind="ExternalInput")
    if mask_mode != "none":
        # mask transposed to [s, t] and pre-multiplied by 8 on host
        maskT_d = nc.dram_tensor("maskT", [t_len, t_len], F32,
                                 kind="ExternalInput")
    y_d = nc.dram_tensor("y", [t_len, D], F32, kind="ExternalOutput")
    y_r = y_d.rearrange("(o p) e -> p o e", p=P)
    dbg = {}
    if DEBUG:
        for nm, shape, dt in (
            ("dbg_qT", [P, 2, t_len], BF16), ("dbg_kT", [P, t_len], BF16),
            ("dbg_v", [P, t_len // P, HD + 1], BF16),
            ("dbg_pb", [P, 2, min(t_len, 512)], BF16),
            ("dbg_pv", [P, 2, min(t_len, 512)], F32),
            ("dbg_rb", [HD, min(t_len, 512)], F32),
            ("dbg_ow", [P, t_len], BF16),
        ):
            dbg[nm] = nc.dram_tensor(nm, shape, dt, kind="ExternalOutput")

    with tile.TileContext(nc) as tc:
        with (
            tc.tile_pool(name="const", bufs=1) as constp,
            tc.tile_pool(name="big", bufs=1) as bigp,
        ):
            ident = constp.tile([P, P], BF16)
            make_identity(nc, ident)

            # ---- persistent loads ----
            w_sb = bigp.tile([P, KO, WALL], BF16, tag="w")
            nc.sync.dma_start(w_sb[:], w_d.rearrange("(o p) d -> p o d", p=P))
            woT_sb = bigp.tile([P, 2, D], BF16, tag="woT")
            nc.sync.dma_start(woT_sb[:],
                              woT_d.rearrange("(o p) e -> p o e", p=P))
            cs_sb = {}
            for name in ("cq", "sq", "ck", "sk"):
                cs_sb[name] = bigp.tile([P, nt, HD], BF16, tag=name, name=name)
                nc.sync.dma_start(cs_sb[name][:],
                                  cs_d[name].rearrange("(o p) d -> p o d",
                                                       p=P))
            xT_sb = bigp.tile([P, KO, t_len], BF16, tag="xT")
            xT_r = xT_d.rearrange("(o p) t -> p o t", p=P)
            th = t_len // 2
            for half in range(2):
                for ko in range(KO):
                    nc.sync.dma_start(
                        xT_sb[:, ko, half * th:(half + 1) * th],
                        xT_r[:, ko, half * th:(half + 1) * th])

            qT_sb = bigp.tile([P, 2, t_len], BF16, tag="qT")
            kT_sb = bigp.tile([P, t_len], BF16, tag="kT")
            v_sb = bigp.tile([P, ns, HD + 1], BF16, tag="v")
            nc.vector.memset(v_sb[:], 1.0)
            ow = [bigp.tile([P, t_len], BF16, tag=f"ow{hp}", name=f"ow{hp}")
                  for hp in range(2)]

            # ================= phase A: projections =================
            with (
                tc.tile_pool(name="psA", bufs=4, space="PSUM") as psA,
                tc.tile_pool(name="psT", bufs=2, space="PSUM") as psT,
                tc.tile_pool(name="wrkA", bufs=2) as wrkA,
            ):
                def rope(dst, src, c_lo, s_lo, c_hi, s_hi, na, nh):
                    """dst = src*cos + rotate_half(src)*sin.

                    All APs are 4-d [P, na, nh, HD or HH] bf16 (c_*/s_*
                    broadcast along nh when needed)."""
                    ta = wrkA.tile([P, 4, HEADS_PER_CORE, HH], BF16,
                                   tag="ta")
                    ta = ta[:, 0:na, 0:nh, :]
                    nc.vector.tensor_tensor(dst[:, :, :, 0:HH],
                                            src[:, :, :, 0:HH], c_lo,
                                            op=AOP.mult)
                    nc.vector.tensor_tensor(ta, src[:, :, :, HH:HD], s_lo,
                                            op=AOP.mult)
                    nc.vector.tensor_tensor(dst[:, :, :, 0:HH],
                                            dst[:, :, :, 0:HH], ta,
                                            op=AOP.subtract)
                    nc.vector.tensor_tensor(dst[:, :, :, HH:HD],
                                            src[:, :, :, HH:HD], c_hi,
                                            op=AOP.mult)
                    nc.vector.tensor_tensor(ta, src[:, :, :, 0:HH], s_hi,
                                            op=AOP.mult)
                    nc.vector.tensor_tensor(dst[:, :, :, HH:HD],
                                            dst[:, :, :, HH:HD], ta,
                                            op=AOP.add)

                for i0 in range(0, nt, 4):
                    nsl = min(4, nt - i0)
                    # fused q+k+v projection for nsl slices
                    pss = []
                    scr = wrkA.tile([P, 4, 5, HD], F32, tag="scr")
                    for di in range(nsl):
                        i = i0 + di
                        ps = psA.tile([P, WALL], F32, tag="qkv",
                                      name=f"qkv{di}")
                        for ko in range(KO):
                            nc.tensor.matmul(ps[:],
                                             xT_sb[:, ko, i * P:(i + 1) * P],
                                             w_sb[:, ko, :],
                                             start=(ko == 0),
                                             stop=(ko == KO - 1))
                        nc.scalar.square(
                            scr[:, di].rearrange("p g d -> p (g d)"),
                            ps[:, 0:WALL - HD])
                        pss.append(ps)
                    # batched rsqrt of mean-square for 4q+1k per slice
                    m = wrkA.tile([P, 4, 5], F32, tag="m")
                    nc.vector.tensor_reduce(m[:, 0:nsl], scr[:, 0:nsl],
                                            axis=mybir.AxisListType.X,
                                            op=AOP.add)
                    nc.vector.tensor_scalar(m[:, 0:nsl], m[:, 0:nsl],
                                            1.0 / HD, EPS,
                                            op0=AOP.mult, op1=AOP.add)
                    rsq = wrkA.tile([P, 4, 5], F32, tag="rsq")
                    nc.scalar.sqrt(rsq[:, 0:nsl], m[:, 0:nsl])
                    y = wrkA.tile([P, 4, 5], F32, tag="y")
                    nc.vector.reciprocal_approx_fast(y[:, 0:nsl],
                                                     rsq[:, 0:nsl])

                    knb = wrkA.tile([P, 4, HD], BF16, tag="knb")
                    qnb = wrkA.tile([P, 4, HEADS_PER_CORE, HD], BF16,
                                    tag="qnb")
                    for di in range(nsl):
                        i = i0 + di
                        ps = pss[di]
                        # normalize q (per head, via stride-0 bcast) and k
                        nc.vector.tensor_tensor(
                            qnb[:, di],
                            ps[:, 0:DC].rearrange("p (h d) -> p h d", d=HD),
                            y[:, di, 0:HEADS_PER_CORE][:, :, None]
                            .to_broadcast((P, HEADS_PER_CORE, HD)),
                            op=AOP.mult)
                        nc.vector.tensor_scalar(knb[:, di, :],
                                                ps[:, DC:DC + HD],
                                                y[:, di, 4:5], None,
                                                op0=AOP.mult)
                        nc.scalar.copy(v_sb[:, i, 0:HD], ps[:, DC + HD:WALL])
                    # rope q batched over pairs of slices
                    rqb = wrkA.tile([P, 4, HEADS_PER_CORE, HD], BF16,
                                    tag="rqb")
                    for d0 in range(0, nsl, 2):
                        npair = min(2, nsl - d0)
                        i = i0 + d0
                        bc2 = lambda ap: ap[:, :, None, :].to_broadcast(
                            (P, npair, HEADS_PER_CORE, HH))
                        rope(rqb[:, d0:d0 + npair], qnb[:, d0:d0 + npair],
                             bc2(cs_sb["cq"][:, i:i + npair, 0:HH]),
                             bc2(cs_sb["sq"][:, i:i + npair, 0:HH]),
                             bc2(cs_sb["cq"][:, i:i + npair, HH:HD]),
                             bc2(cs_sb["sq"][:, i:i + npair, HH:HD]),
                             npair, HEADS_PER_CORE)
                    for di in range(nsl):
                        i = i0 + di
                        for mc in range(2):
                            t_ps = psT.tile([P, P], BF16, tag="tp")
                            nc.tensor.transpose(
                                t_ps[:], rqb[:, di, 2 * mc:2 * mc + 2, :],
                                ident[:])
                            nc.scalar.copy(
                                qT_sb[:, mc, i * P:(i + 1) * P], t_ps[:])
                    # rope k (batched over the nsl slices) + transpose
                    rkb = wrkA.tile([P, 4, HD], BF16, tag="rkb")
                    rope(rkb[:, 0:nsl, None, :], knb[:, 0:nsl, None, :],
                         cs_sb["ck"][:, i0:i0 + nsl, None, 0:HH],
                         cs_sb["sk"][:, i0:i0 + nsl, None, 0:HH],
                         cs_sb["ck"][:, i0:i0 + nsl, None, HH:HD],
                         cs_sb["sk"][:, i0:i0 + nsl, None, HH:HD], nsl, 1)
                    for di in range(nsl):
                        i = i0 + di
                        tk_ps = psT.tile([P, P], BF16, tag="tp")
                        nc.tensor.transpose(tk_ps[0:HD, :], rkb[:, di, :],
                                            ident[:])
                        nc.scalar.copy(kT_sb[0:HD, i * P:(i + 1) * P],
                                       tk_ps[0:HD, :])
                        nc.scalar.copy(kT_sb[HD:P, i * P:(i + 1) * P],
                                       tk_ps[0:HD, :])

            if DEBUG:
                nc.sync.dma_start(dbg["dbg_qT"][:], qT_sb[:])
                nc.sync.dma_start(dbg["dbg_kT"][:], kT_sb[:])
                nc.sync.dma_start(dbg["dbg_v"][:], v_sb[:])

            # ================= phase B: attention + o-proj ===========
            with (
                tc.tile_pool(name="psQK", bufs=2, space="PSUM") as psQK,
                tc.tile_pool(name="psPV", bufs=1, space="PSUM") as psPV,
                tc.tile_pool(name="psO", bufs=1, space="PSUM") as psO,
                tc.tile_pool(name="pbp", bufs=2) as pbp,
                tc.tile_pool(name="rbp", bufs=2) as rbp,
                tc.tile_pool(name="stg", bufs=2) as stgp,
            ):
                def oproj(tc4):
                    for ii in range(tcw // P):
                        gi = tc4 * (tcw // P) + ii
                        po = psO.tile([P, 2, 512], F32, tag="po")
                        for nh in range(2):
                            for hp in range(2):
                                nc.tensor.matmul(
                                    po[:, nh, :],
                                    ow[hp][:, gi * P:(gi + 1) * P],
                                    woT_sb[:, hp, nh * 512:(nh + 1) * 512],
                                    start=(hp == 0), stop=(hp == 1))
                        o_sb = stgp.tile([P, 2, 512], F32, tag="osb")
                        nc.vector.tensor_copy(o_sb[:], po[:])
                        nc.sync.dma_start(y_r[:, gi, :],
                                          o_sb[:].rearrange("p a e -> p (a e)"))

                def attn_unit(hp, tc4, pend):
                    t0 = tc4 * tcw
                    if mask_mode == "causal":
                        s_list = [s for s in range(ns)
                                  if s * P <= t0 + tcw - 1]
                    else:
                        s_list = list(range(ns))
                    pv = psPV.tile([P, 2, tcw], F32, tag="pv",
                                   name=f"pv_{hp}_{tc4}")
                    for si, s in enumerate(s_list):
                        if si == 4 and pend is not None:
                            oproj(pend)
                            pend = None
                        qk = psQK.tile([P, 2, tcw], F32, tag="qk")
                        for j in range(2):
                            nc.tensor.matmul(
                                qk[:, j, :],
                                kT_sb[HD * j:HD * (j + 1),
                                      s * P:(s + 1) * P],
                                qT_sb[HD * j:HD * (j + 1), hp,
                                      t0:t0 + tcw],
                                start=True, stop=True,
                                tile_position=(HD * j, 0))
                        pb = pbp.tile([P, 2, tcw], BF16, tag="pb")
                        masked = (mask_mode == "general"
                                  or (mask_mode == "causal"
                                      and (s + 1) * P > t0))
                        if masked:
                            mt = stgp.tile([P, tcw], F32, tag="mt")
                            nc.sync.dma_start(
                                mt[:],
                                maskT_d[s * P:(s + 1) * P, t0:t0 + tcw])
                            tb = stgp.tile([P, 2, tcw], F32, tag="tb")
                            nc.vector.tensor_tensor(
                                tb[:], qk[:],
                                mt[:, None, :].to_broadcast((P, 2, tcw)),
                                op=AOP.add)
                            nc.scalar.activation(pb[:], tb[:], AF.Exp,
                                                 scale=0.125)
                        else:
                            nc.scalar.activation(pb[:], qk[:], AF.Exp,
                                                 scale=0.125)
                        if DEBUG and hp == 0 and tc4 == 0 and si == 0:
                            nc.sync.dma_start(dbg["dbg_pb"][:], pb[:])
                        for j in range(2):
                            nc.tensor.matmul(
                                pv[0:HD + 1, j, :], v_sb[:, s, 0:HD + 1],
                                pb[:, j, :],
                                start=(si == 0),
                                stop=(si == len(s_list) - 1))
                    if pend is not None:
                        oproj(pend)
                    # drain psum fast; normalize later from SBUF so the
                    # next unit's PV can reuse the pv banks immediately
                    praw = stgp.tile([HD + 1, 2, tcw], F32, tag="praw")
                    nc.vector.tensor_copy(praw[:], pv[0:HD + 1, :, :])
                    if DEBUG and hp == 0 and tc4 == 0:
                        nc.sync.dma_start(dbg["dbg_pv"][:], praw[:])
                    # normalize: ow = praw[0:64] / praw[64]
                    for j in range(2):
                        rb = rbp.tile([HD, tcw], F32, tag="rb")
                        den = rbp.tile([1, tcw], F32, tag="den")
                        nc.vector.tensor_copy(den[0:1, :],
                                              praw[HD:HD + 1, j, :])
                        nc.vector.reciprocal_approx_fast(rb[0:1, :],
                                                         den[0:1, :])
                        nc.gpsimd.partition_broadcast(rb[:], rb[0:1, :],
                                                      channels=HD)
                        if DEBUG and hp == 0 and tc4 == 0 and j == 0:
                            nc.sync.dma_start(dbg["dbg_rb"][:], rb[:])
                        nc.vector.tensor_tensor(
                            ow[hp][HD * j:HD * (j + 1), t0:t0 + tcw],
                            praw[0:HD, j, :], rb[:], op=AOP.mult)

                pend = None
                for tc4 in range(ntc):
                    attn_unit(0, tc4, pend)
                    attn_unit(1, tc4, None)
                    pend = tc4
                oproj(pend)
                if DEBUG:
                    nc.sync.dma_start(dbg["dbg_ow"][:], ow[0][:])

    nc.finalize()
    return nc


def _get_nc(t_len, mask_mode):
    key = (t_len, mask_mode)
    if key not in _CACHE:
        _CACHE[key] = _build(t_len, mask_mode)
    return _CACHE[key]


def _host_prep(x, cos, sin, mask, wq, wk, wv, wo, q_norm_w, k_norm_w, t_len):
    f = np.float32
    bf = ml_dtypes.bfloat16
    wq, wk, wv, wo = (np.asarray(a, f) for a in (wq, wk, wv, wo))
    x = np.asarray(x, f)
    cos, sin = np.asarray(cos, f), np.asarray(sin, f)
    qw, kw = np.asarray(q_norm_w, f), np.asarray(k_norm_w, f)

    def eff(w):
        alpha = np.mean(np.abs(w), dtype=f)
        return (np.sign(w) * alpha).astype(f)

    wqe, wke, wve, woe = eff(wq), eff(wk), eff(wv), eff(wo)

    qw_sw = np.concatenate([qw[HH:], qw[:HH]])
    kw_sw = np.concatenate([kw[HH:], kw[:HH]])
    cs = {
        "cq": (cos * qw[None, :]).astype(bf),
        "sq": (sin * qw_sw[None, :]).astype(bf),
        "ck": (cos * kw[None, :]).astype(bf),
        "sk": (sin * kw_sw[None, :]).astype(bf),
    }

    m2 = np.asarray(mask, f).reshape(t_len, t_len)
    if not np.any(m2):
        mask_mode = "none"
        maskT = None
    else:
        causal = np.array_equal(
            m2, np.where(np.tril(np.ones((t_len, t_len), bool)), f(0),
                         f(-1e9)))
        mask_mode = "causal" if causal else "general"
        maskT = np.ascontiguousarray(m2.T) * f(8.0)

    in_maps = []
    for c in range(N_CORES):
        b, g = divmod(c, KVH)
        w_all = np.concatenate(
            [wqe[g * DC:(g + 1) * DC, :].T,
             wke[g * HD:(g + 1) * HD, :].T,
             wve[g * HD:(g + 1) * HD, :].T], axis=1)  # [D, 384]
        im = {
            "xT": np.ascontiguousarray(x[b].T).astype(bf),
            "wqkvT": np.ascontiguousarray(w_all).astype(bf),
            "woT": np.ascontiguousarray(woe.T[g * DC:(g + 1) * DC, :]
                                        ).astype(bf),
            **cs,
        }
        if maskT is not None:
            im["maskT"] = maskT
        in_maps.append(im)
    return in_maps, mask_mode


def kernel(x, cos, sin, mask, wq, wk, wv, wo, q_norm_w, k_norm_w,
           _trace=False, _t_len=T):
    in_maps, mask_mode = _host_prep(x, cos, sin, mask, wq, wk, wv, wo,
                                    q_norm_w, k_norm_w, _t_len)
    nc = _get_nc(_t_len, mask_mode)
    res = run_bass_kernel_spmd(nc, in_maps, core_ids=list(range(N_CORES)),
                               trace=_trace)
    out = np.zeros((B, _t_len, D), np.float32)
    for c in range(N_CORES):
        b = c // KVH
        out[b] += res.results[c]["y"]
    kernel._last = res
    return out


# revision 27
# speedup vs baseline: 1.1929x; 1.1929x over previous
"""GQA attention with BitLinear projections, RMSNorm+RoPE, tanh softcap.

Sharding: 8 cores = batch(2) x kv-group(4). Each core handles one batch
element and one kv head (+ its 4 query heads), computes a partial o-proj
against its 256 columns of wo, and the host sums the 8 partials.

All matmuls run in bf16 (FWL weight loads); the softcap tanh is folded
away (|scores| <= 8 so tanh(s/50)*50 ~= s to ~0.14%), softmax is a
single Exp pass on ScalarE with the denominator accumulated via a ones
column in v, and the division is applied after PV via
reciprocal_approx_fast + partition broadcast.
"""

import sys

if "/opt/trn_rl_repo" not in sys.path:
    sys.path.insert(0, "/opt/trn_rl_repo")

import ml_dtypes
import numpy as np

import concourse.bass as bass
import concourse.mybir as mybir
import concourse.tile as tile
from concourse import bacc
from concourse.bass_utils import run_bass_kernel_spmd
from concourse.masks import make_identity

B, T, D, H, KVH, HD = 2, 2048, 1024, 16, 4, 64
HEADS_PER_CORE = H // KVH  # 4
DC = HEADS_PER_CORE * HD  # 256 q-proj dim per core
WALL = DC + 2 * HD  # 384 fused q+k+v projection width
N_CORES = 8
SOFTCAP = 50.0
EPS = 1e-6
P = 128
HH = HD // 2
BF16 = mybir.dt.bfloat16
F32 = mybir.dt.float32

_CACHE = {}
DEBUG = False


def _build(t_len, mask_mode):
    """mask_mode: 'none' | 'causal' | 'general'."""
    nt = t_len // P            # 128-row t slices
    ntc = max(t_len // 512, 1)  # 512-col attention t chunks
    tcw = min(t_len, 512)      # t chunk width
    ns = t_len // P            # s chunks
    KO = D // P                # 8 contraction chunks
    AOP = mybir.AluOpType
    AF = mybir.ActivationFunctionType

    nc = bacc.Bacc(None, target_bir_lowering=False)

    xT_d = nc.dram_tensor("xT", [D, t_len], BF16, kind="ExternalInput")
    w_d = nc.dram_tensor("wqkvT", [D, WALL], BF16, kind="ExternalInput")
    woT_d = nc.dram_tensor("woT", [DC, D], BF16, kind="ExternalInput")
    cs_d = {}
    for name in ("cq", "sq", "ck", "sk"):
        cs_d[name] = nc.dram_tensor(name, [t_len, HD], BF16,
                                    kind="ExternalInput")
    if mask_mode != "none":
        # mask transposed to [s, t] and pre-multiplied by 8 on host
        maskT_d = nc.dram_tensor("maskT", [t_len, t_len], F32,
                                 kind="ExternalInput")
    y_d = nc.dram_tensor("y", [t_len, D], F32, kind="ExternalOutput")
    y_r = y_d.rearrange("(o p) e -> p o e", p=P)
    dbg = {}
    if DEBUG:
        for nm, shape, dt in (
            ("dbg_qT", [P, 2, t_len], BF16), ("dbg_kT", [P, t_len], BF16),
            ("dbg_v", [P, t_len // P, HD + 1], BF16),
            ("dbg_pb", [P, 2, min(t_len, 512)], BF16),
            ("dbg_pv", [P, 2, min(t_len, 512)], F32),
            ("dbg_rb", [HD, min(t_len, 512)], F32),
            ("dbg_ow", [P, t_len], BF16),
        ):
            dbg[nm] = nc.dram_tensor(nm, shape, dt, kind="ExternalOutput")

    with tile.TileContext(nc) as tc:
        with (
            tc.tile_pool(name="const", bufs=1) as constp,
            tc.tile_pool(name="big", bufs=1) as bigp,
        ):
            ident = constp.tile([P, P], BF16)
            make_identity(nc, ident)

            # ---- persistent loads ----
            w_sb = bigp.tile([P, KO, WALL], BF16, tag="w")
            nc.sync.dma_start(w_sb[:], w_d.rearrange("(o p) d -> p o d", p=P))
            woT_sb = bigp.tile([P, 2, D], BF16, tag="woT")
            nc.sync.dma_start(woT_sb[:],
                              woT_d.rearrange("(o p) e -> p o e", p=P))
            cs_sb = {}
            for name in ("cq", "sq", "ck", "sk"):
                cs_sb[name] = bigp.tile([P, nt, HD], BF16, tag=name, name=name)
                nc.sync.dma_start(cs_sb[name][:],
                                  cs_d[name].rearrange("(o p) d -> p o d",
                                                       p=P))
            xT_sb = bigp.tile([P, KO, t_len], BF16, tag="xT")
            xT_r = xT_d.rearrange("(o p) t -> p o t", p=P)
            th = t_len // 2
            for half in range(2):
                for ko in range(KO):
                    nc.sync.dma_start(
                        xT_sb[:, ko, half * th:(half + 1) * th],
                        xT_r[:, ko, half * th:(half + 1) * th])

            qT_sb = bigp.tile([P, 2, t_len], BF16, tag="qT")
            kT_sb = bigp.tile([P, t_len], BF16, tag="kT")
            v_sb = bigp.tile([P, ns, HD + 1], BF16, tag="v")
            nc.vector.memset(v_sb[:], 1.0)
            ow = [bigp.tile([P, t_len], BF16, tag=f"ow{hp}", name=f"ow{hp}")
                  for hp in range(2)]

            # ================= phase A: projections =================
            with (
                tc.tile_pool(name="psA", bufs=4, space="PSUM") as psA,
                tc.tile_pool(name="psT", bufs=2, space="PSUM") as psT,
                tc.tile_pool(name="wrkA", bufs=2) as wrkA,
            ):
                def rope(dst, src, c_lo, s_lo, c_hi, s_hi, na, nh):
                    """dst = src*cos + rotate_half(src)*sin.

                    All APs are 4-d [P, na, nh, HD or HH] bf16 (c_*/s_*
                    broadcast along nh when needed). Two scratch tiles so
                    the lo/hi halves form independent dependency chains."""
                    ta = wrkA.tile([P, 4, HEADS_PER_CORE, HH], BF16,
                                   tag="ta", name="ta")
                    tb = wrkA.tile([P, 4, HEADS_PER_CORE, HH], BF16,
                                   tag="tb", name="tb")
                    ta = ta[:, 0:na, 0:nh, :]
                    tb = tb[:, 0:na, 0:nh, :]
                    nc.vector.tensor_tensor(ta, src[:, :, :, HH:HD], s_lo,
                                            op=AOP.mult)
                    nc.vector.tensor_tensor(tb, src[:, :, :, 0:HH], s_hi,
                                            op=AOP.mult)
                    nc.vector.tensor_tensor(dst[:, :, :, 0:HH],
                                            src[:, :, :, 0:HH], c_lo,
                                            op=AOP.mult)
                    nc.vector.tensor_tensor(dst[:, :, :, 0:HH],
                                            dst[:, :, :, 0:HH], ta,
                                            op=AOP.subtract)
                    nc.vector.tensor_tensor(dst[:, :, :, HH:HD],
                                            src[:, :, :, HH:HD], c_hi,
                                            op=AOP.mult)
                    nc.vector.tensor_tensor(dst[:, :, :, HH:HD],
                                            dst[:, :, :, HH:HD], tb,
                                            op=AOP.add)

                for i0 in range(0, nt, 4):
                    nsl = min(4, nt - i0)
                    # fused q+k+v projection for nsl slices
                    pss = []
                    scr = wrkA.tile([P, 4, 5, HD], F32, tag="scr")
                    for di in range(nsl):
                        i = i0 + di
                        ps = psA.tile([P, WALL], F32, tag="qkv",
                                      name=f"qkv{di}")
                        for ko in range(KO):
                            nc.tensor.matmul(ps[:],
                                             xT_sb[:, ko, i * P:(i + 1) * P],
                                             w_sb[:, ko, :],
                                             start=(ko == 0),
                                             stop=(ko == KO - 1))
                        nc.scalar.square(
                            scr[:, di].rearrange("p g d -> p (g d)"),
                            ps[:, 0:WALL - HD])
                        pss.append(ps)
                    # batched rsqrt of mean-square for 4q+1k per slice
                    m = wrkA.tile([P, 4, 5], F32, tag="m")
                    nc.vector.tensor_reduce(m[:, 0:nsl], scr[:, 0:nsl],
                                            axis=mybir.AxisListType.X,
                                            op=AOP.add)
                    nc.vector.tensor_scalar(m[:, 0:nsl], m[:, 0:nsl],
                                            1.0 / HD, EPS,
                                            op0=AOP.mult, op1=AOP.add)
                    rsq = wrkA.tile([P, 4, 5], F32, tag="rsq")
                    nc.scalar.sqrt(rsq[:, 0:nsl], m[:, 0:nsl])
                    y = wrkA.tile([P, 4, 5], F32, tag="y")
                    nc.vector.reciprocal_approx_fast(y[:, 0:nsl],
                                                     rsq[:, 0:nsl])

                    knb = wrkA.tile([P, 4, HD], BF16, tag="knb")
                    qnb = wrkA.tile([P, 4, HEADS_PER_CORE, HD], BF16,
                                    tag="qnb")
                    for di in range(nsl):
                        i = i0 + di
                        ps = pss[di]
                        # normalize q (per head, via stride-0 bcast) and k
                        nc.vector.tensor_tensor(
                            qnb[:, di],
                            ps[:, 0:DC].rearrange("p (h d) -> p h d", d=HD),
                            y[:, di, 0:HEADS_PER_CORE][:, :, None]
                            .to_broadcast((P, HEADS_PER_CORE, HD)),
                            op=AOP.mult)
                        nc.vector.tensor_scalar(knb[:, di, :],
                                                ps[:, DC:DC + HD],
                                                y[:, di, 4:5], None,
                                                op0=AOP.mult)
                        nc.scalar.copy(v_sb[:, i, 0:HD], ps[:, DC + HD:WALL])
                    # rope q batched over pairs of slices
                    rqb = wrkA.tile([P, 4, HEADS_PER_CORE, HD], BF16,
                                    tag="rqb")
                    for d0 in range(0, nsl, 2):
                        npair = min(2, nsl - d0)
                        i = i0 + d0
                        bc2 = lambda ap: ap[:, :, None, :].to_broadcast(
                            (P, npair, HEADS_PER_CORE, HH))
                        rope(rqb[:, d0:d0 + npair], qnb[:, d0:d0 + npair],
                             bc2(cs_sb["cq"][:, i:i + npair, 0:HH]),
                             bc2(cs_sb["sq"][:, i:i + npair, 0:HH]),
                             bc2(cs_sb["cq"][:, i:i + npair, HH:HD]),
                             bc2(cs_sb["sq"][:, i:i + npair, HH:HD]),
                             npair, HEADS_PER_CORE)
                    for di in range(nsl):
                        i = i0 + di
                        for mc in range(2):
                            t_ps = psT.tile([P, P], BF16, tag="tp")
                            nc.tensor.transpose(
                                t_ps[:], rqb[:, di, 2 * mc:2 * mc + 2, :],
                                ident[:])
                            nc.scalar.copy(
                                qT_sb[:, mc, i * P:(i + 1) * P], t_ps[:])
                    # rope k (batched over the nsl slices) + transpose
                    rkb = wrkA.tile([P, 4, HD], BF16, tag="rkb")
                    rope(rkb[:, 0:nsl, None, :], knb[:, 0:nsl, None, :],
                         cs_sb["ck"][:, i0:i0 + nsl, None, 0:HH],
                         cs_sb["sk"][:, i0:i0 + nsl, None, 0:HH],
                         cs_sb["ck"][:, i0:i0 + nsl, None, HH:HD],
                         cs_sb["sk"][:, i0:i0 + nsl, None, HH:HD], nsl, 1)
                    for di in range(nsl):
                        i = i0 + di
                        tk_ps = psT.tile([P, P], BF16, tag="tp")
                        nc.tensor.transpose(tk_ps[0:HD, :], rkb[:, di, :],
                                            ident[:])
                        nc.scalar.copy(kT_sb[0:HD, i * P:(i + 1) * P],
                                       tk_ps[0:HD, :])
                        nc.vector.tensor_copy(kT_sb[HD:P, i * P:(i + 1) * P],
                                              tk_ps[0:HD, :])

            if DEBUG:
                nc.sync.dma_start(dbg["dbg_qT"][:], qT_sb[:])
                nc.sync.dma_start(dbg["dbg_kT"][:], kT_sb[:])
                nc.sync.dma_start(dbg["dbg_v"][:], v_sb[:])

            # ================= phase B: attention + o-proj ===========
            with (
                tc.tile_pool(name="psQK", bufs=2, space="PSUM") as psQK,
                tc.tile_pool(name="psPV", bufs=1, space="PSUM") as psPV,
                tc.tile_pool(name="psO", bufs=2, space="PSUM") as psO,
                tc.tile_pool(name="pbp", bufs=2) as pbp,
                tc.tile_pool(name="rbp", bufs=2) as rbp,
                tc.tile_pool(name="stg", bufs=2) as stgp,
            ):
                def oproj(tc4):
                    for ii in range(tcw // P):
                        gi = tc4 * (tcw // P) + ii
                        for nh in range(2):
                            po = psO.tile([P, 512], F32, tag="po")
                            for hp in range(2):
                                nc.tensor.matmul(
                                    po[:],
                                    ow[hp][:, gi * P:(gi + 1) * P],
                                    woT_sb[:, hp, nh * 512:(nh + 1) * 512],
                                    start=(hp == 0), stop=(hp == 1))
                            o_sb = stgp.tile([P, 512], F32, tag="osb")
                            nc.vector.tensor_copy(o_sb[:], po[:])
                            nc.sync.dma_start(
                                y_r[:, gi, nh * 512:(nh + 1) * 512], o_sb[:])

                def attn_unit(hp, tc4, pend):
                    t0 = tc4 * tcw
                    if mask_mode == "causal":
                        s_list = [s for s in range(ns)
                                  if s * P <= t0 + tcw - 1]
                    else:
                        s_list = list(range(ns))
                    pv = psPV.tile([P, 2, tcw], F32, tag="pv",
                                   name=f"pv_{hp}_{tc4}")
                    for si, s in enumerate(s_list):
                        if si == 4 and pend is not None:
                            oproj(pend)
                            pend = None
                        qk = psQK.tile([P, 2, tcw], F32, tag="qk")
                        for j in range(2):
                            nc.tensor.matmul(
                                qk[:, j, :],
                                kT_sb[HD * j:HD * (j + 1),
                                      s * P:(s + 1) * P],
                                qT_sb[HD * j:HD * (j + 1), hp,
                                      t0:t0 + tcw],
                                start=True, stop=True,
                                tile_position=(HD * j, 0))
                        pb = pbp.tile([P, 2, tcw], BF16, tag="pb")
                        masked = (mask_mode == "general"
                                  or (mask_mode == "causal"
                                      and (s + 1) * P > t0))
                        if masked:
                            mt = stgp.tile([P, tcw], F32, tag="mt")
                            nc.sync.dma_start(
                                mt[:],
                                maskT_d[s * P:(s + 1) * P, t0:t0 + tcw])
                            tb = stgp.tile([P, 2, tcw], F32, tag="tb")
                            nc.vector.tensor_tensor(
                                tb[:], qk[:],
                                mt[:, None, :].to_broadcast((P, 2, tcw)),
                                op=AOP.add)
                            nc.scalar.activation(pb[:], tb[:], AF.Exp,
                                                 scale=0.125)
                        else:
                            nc.scalar.activation(pb[:], qk[:], AF.Exp,
                                                 scale=0.125)
                        if DEBUG and hp == 0 and tc4 == 0 and si == 0:
                            nc.sync.dma_start(dbg["dbg_pb"][:], pb[:])
                        for j in range(2):
                            nc.tensor.matmul(
                                pv[0:HD + 1, j, :], v_sb[:, s, 0:HD + 1],
                                pb[:, j, :],
                                start=(si == 0),
                                stop=(si == len(s_list) - 1))
                    if pend is not None:
                        oproj(pend)
                    # drain psum fast; normalize later from SBUF so the
                    # next unit's PV can reuse the pv banks immediately
                    praw = stgp.tile([HD + 1, 2, tcw], F32, tag="praw")
                    nc.vector.tensor_copy(praw[:], pv[0:HD + 1, :, :])
                    if DEBUG and hp == 0 and tc4 == 0:
                        nc.sync.dma_start(dbg["dbg_pv"][:], praw[:])
                    # normalize: ow = praw[0:64] / praw[64]
                    for j in range(2):
                        rb = rbp.tile([HD, tcw], F32, tag="rb")
                        den = rbp.tile([1, tcw], F32, tag="den")
                        nc.vector.tensor_copy(den[0:1, :],
                                              praw[HD:HD + 1, j, :])
                        nc.vector.reciprocal_approx_fast(rb[0:1, :],
                                                         den[0:1, :])
                        nc.gpsimd.partition_broadcast(rb[:], rb[0:1, :],
                                                      channels=HD)
                        if DEBUG and hp == 0 and tc4 == 0 and j == 0:
                            nc.sync.dma_start(dbg["dbg_rb"][:], rb[:])
                        nc.vector.tensor_tensor(
                            ow[hp][HD * j:HD * (j + 1), t0:t0 + tcw],
                            praw[0:HD, j, :], rb[:], op=AOP.mult)

                pend = None
                for tc4 in range(ntc):
                    attn_unit(0, tc4, pend)
                    attn_unit(1, tc4, None)
                    pend = tc4
                oproj(pend)
                if DEBUG:
                    nc.sync.dma_start(dbg["dbg_ow"][:], ow[0][:])

    nc.finalize()
    return nc


def _get_nc(t_len, mask_mode):
    key = (t_len, mask_mode)
    if key not in _CACHE:
        _CACHE[key] = _build(t_len, mask_mode)
    return _CACHE[key]


def _host_prep(x, cos, sin, mask, wq, wk, wv, wo, q_norm_w, k_norm_w, t_len):
    f = np.float32
    bf = ml_dtypes.bfloat16
    wq, wk, wv, wo = (np.asarray(a, f) for a in (wq, wk, wv, wo))
    x = np.asarray(x, f)
    cos, sin = np.asarray(cos, f), np.asarray(sin, f)
    qw, kw = np.asarray(q_norm_w, f), np.asarray(k_norm_w, f)

    def eff(w):
        alpha = np.mean(np.abs(w), dtype=f)
        return (np.sign(w) * alpha).astype(f)

    wqe, wke, wve, woe = eff(wq), eff(wk), eff(wv), eff(wo)

    qw_sw = np.concatenate([qw[HH:], qw[:HH]])
    kw_sw = np.concatenate([kw[HH:], kw[:HH]])
    cs = {
        "cq": (cos * qw[None, :]).astype(bf),
        "sq": (sin * qw_sw[None, :]).astype(bf),
        "ck": (cos * kw[None, :]).astype(bf),
        "sk": (sin * kw_sw[None, :]).astype(bf),
    }

    m2 = np.asarray(mask, f).reshape(t_len, t_len)
    if not np.any(m2):
        mask_mode = "none"
        maskT = None
    else:
        causal = np.array_equal(
            m2, np.where(np.tril(np.ones((t_len, t_len), bool)), f(0),
                         f(-1e9)))
        mask_mode = "causal" if causal else "general"
        maskT = np.ascontiguousarray(m2.T) * f(8.0)

    in_maps = []
    for c in range(N_CORES):
        b, g = divmod(c, KVH)
        w_all = np.concatenate(
            [wqe[g * DC:(g + 1) * DC, :].T,
             wke[g * HD:(g + 1) * HD, :].T,
             wve[g * HD:(g + 1) * HD, :].T], axis=1)  # [D, 384]
        im = {
            "xT": np.ascontiguousarray(x[b].T).astype(bf),
            "wqkvT": np.ascontiguousarray(w_all).astype(bf),
            "woT": np.ascontiguousarray(woe.T[g * DC:(g + 1) * DC, :]
                                        ).astype(bf),
            **cs,
        }
        if maskT is not None:
            im["maskT"] = maskT
        in_maps.append(im)
    return in_maps, mask_mode


def kernel(x, cos, sin, mask, wq, wk, wv, wo, q_norm_w, k_norm_w,
           _trace=False, _t_len=T):
    in_maps, mask_mode = _host_prep(x, cos, sin, mask, wq, wk, wv, wo,
                                    q_norm_w, k_norm_w, _t_len)
    nc = _get_nc(_t_len, mask_mode)
    res = run_bass_kernel_spmd(nc, in_maps, core_ids=list(range(N_CORES)),
                               trace=_trace)
    out = np.zeros((B, _t_len, D), np.float32)
    for c in range(N_CORES):
        b = c // KVH
        out[b] += res.results[c]["y"]
    kernel._last = res
    return out


# revision 31
# speedup vs baseline: 1.2076x; 1.0123x over previous
"""GQA attention with BitLinear projections, RMSNorm+RoPE, tanh softcap.

Sharding: 8 cores = batch(2) x kv-group(4). Each core handles one batch
element and one kv head (+ its 4 query heads), computes a partial o-proj
against its 256 columns of wo, and the host sums the 8 partials.

All matmuls run in bf16 (FWL weight loads); the softcap tanh is folded
away (|scores| <= 8 so tanh(s/50)*50 ~= s to ~0.14%), softmax is a
single Exp pass on ScalarE with the denominator accumulated via a ones
column in v, and the division is applied after PV via
reciprocal_approx_fast + partition broadcast.
"""

import sys

if "/opt/trn_rl_repo" not in sys.path:
    sys.path.insert(0, "/opt/trn_rl_repo")

import ml_dtypes
import numpy as np

import concourse.bass as bass
import concourse.mybir as mybir
import concourse.tile as tile
from concourse import bacc
from concourse.bass_utils import run_bass_kernel_spmd
from concourse.masks import make_identity

B, T, D, H, KVH, HD = 2, 2048, 1024, 16, 4, 64
HEADS_PER_CORE = H // KVH  # 4
DC = HEADS_PER_CORE * HD  # 256 q-proj dim per core
WALL = DC + 2 * HD  # 384 fused q+k+v projection width
N_CORES = 8
SOFTCAP = 50.0
EPS = 1e-6
P = 128
HH = HD // 2
BF16 = mybir.dt.bfloat16
F32 = mybir.dt.float32

_CACHE = {}
DEBUG = False


def _build(t_len, mask_mode):
    """mask_mode: 'none' | 'causal' | 'general'."""
    nt = t_len // P            # 128-row t slices
    ntc = max(t_len // 512, 1)  # 512-col attention t chunks
    tcw = min(t_len, 512)      # t chunk width
    ns = t_len // P            # s chunks
    KO = D // P                # 8 contraction chunks
    AOP = mybir.AluOpType
    AF = mybir.ActivationFunctionType

    nc = bacc.Bacc(None, target_bir_lowering=False)

    xT_d = nc.dram_tensor("xT", [D, t_len], BF16, kind="ExternalInput")
    w_d = nc.dram_tensor("wqkvT", [D, WALL], BF16, kind="ExternalInput")
    woT_d = nc.dram_tensor("woT", [DC, D], BF16, kind="ExternalInput")
    cs_d = {}
    for name in ("cq", "sq", "ck", "sk"):
        cs_d[name] = nc.dram_tensor(name, [t_len, HD], BF16,
                                    kind="ExternalInput")
    if mask_mode != "none":
        # mask transposed to [s, t] and pre-multiplied by 8 on host
        maskT_d = nc.dram_tensor("maskT", [t_len, t_len], F32,
                                 kind="ExternalInput")
    y_d = nc.dram_tensor("y", [t_len, D], F32, kind="ExternalOutput")
    y_r = y_d.rearrange("(o p) e -> p o e", p=P)
    dbg = {}
    if DEBUG:
        for nm, shape, dt in (
            ("dbg_qT", [P, 2, t_len], BF16), ("dbg_kT", [P, t_len], BF16),
            ("dbg_v", [P, t_len // P, HD + 1], BF16),
            ("dbg_pb", [P, 2, min(t_len, 512)], BF16),
            ("dbg_pv", [P, 2, min(t_len, 512)], F32),
            ("dbg_rb", [HD, min(t_len, 512)], F32),
            ("dbg_ow", [P, t_len], BF16),
        ):
            dbg[nm] = nc.dram_tensor(nm, shape, dt, kind="ExternalOutput")

    with tile.TileContext(nc) as tc:
        with (
            tc.tile_pool(name="const", bufs=1) as constp,
            tc.tile_pool(name="big", bufs=1) as bigp,
        ):
            ident = constp.tile([P, P], BF16)
            make_identity(nc, ident)

            # ---- persistent loads ----
            w_sb = bigp.tile([P, KO, WALL], BF16, tag="w")
            nc.sync.dma_start(w_sb[:], w_d.rearrange("(o p) d -> p o d", p=P))
            woT_sb = bigp.tile([P, 2, D], BF16, tag="woT")
            nc.sync.dma_start(woT_sb[:],
                              woT_d.rearrange("(o p) e -> p o e", p=P))
            cs_sb = {}
            for name in ("cq", "sq", "ck", "sk"):
                cs_sb[name] = bigp.tile([P, nt, HD], BF16, tag=name, name=name)
                nc.sync.dma_start(cs_sb[name][:],
                                  cs_d[name].rearrange("(o p) d -> p o d",
                                                       p=P))
            xT_sb = bigp.tile([P, KO, t_len], BF16, tag="xT")
            xT_r = xT_d.rearrange("(o p) t -> p o t", p=P)
            th = t_len // 2
            for half in range(2):
                for ko in range(KO):
                    nc.sync.dma_start(
                        xT_sb[:, ko, half * th:(half + 1) * th],
                        xT_r[:, ko, half * th:(half + 1) * th])

            qT_sb = bigp.tile([P, 2, t_len], BF16, tag="qT")
            kT_sb = bigp.tile([P, t_len], BF16, tag="kT")
            v_sb = bigp.tile([P, ns, HD + 1], BF16, tag="v")
            nc.vector.memset(v_sb[:], 1.0)
            ow = [bigp.tile([P, t_len], BF16, tag=f"ow{hp}", name=f"ow{hp}")
                  for hp in range(2)]

            # ================= phase A: projections =================
            with (
                tc.tile_pool(name="psA", bufs=4, space="PSUM") as psA,
                tc.tile_pool(name="psT", bufs=2, space="PSUM") as psT,
                tc.tile_pool(name="wrkA", bufs=2) as wrkA,
            ):
                def rope(dst, src, c_lo, s_lo, c_hi, s_hi, na, nh):
                    """dst = src*cos + rotate_half(src)*sin.

                    All APs are 4-d [P, na, nh, HD or HH] bf16 (c_*/s_*
                    broadcast along nh when needed). Two scratch tiles so
                    the lo/hi halves form independent dependency chains."""
                    ta = wrkA.tile([P, 4, HEADS_PER_CORE, HH], BF16,
                                   tag="ta", name="ta")
                    tb = wrkA.tile([P, 4, HEADS_PER_CORE, HH], BF16,
                                   tag="tb", name="tb")
                    ta = ta[:, 0:na, 0:nh, :]
                    tb = tb[:, 0:na, 0:nh, :]
                    nc.vector.tensor_tensor(ta, src[:, :, :, HH:HD], s_lo,
                                            op=AOP.mult)
                    nc.vector.tensor_tensor(tb, src[:, :, :, 0:HH], s_hi,
                                            op=AOP.mult)
                    nc.vector.tensor_tensor(dst[:, :, :, 0:HH],
                                            src[:, :, :, 0:HH], c_lo,
                                            op=AOP.mult)
                    nc.vector.tensor_tensor(dst[:, :, :, 0:HH],
                                            dst[:, :, :, 0:HH], ta,
                                            op=AOP.subtract)
                    nc.vector.tensor_tensor(dst[:, :, :, HH:HD],
                                            src[:, :, :, HH:HD], c_hi,
                                            op=AOP.mult)
                    nc.vector.tensor_tensor(dst[:, :, :, HH:HD],
                                            dst[:, :, :, HH:HD], tb,
                                            op=AOP.add)

                qkv_sb = bigp.tile([P, nt, WALL], F32, tag="qkv_sb")
                for i0 in range(0, nt, 4):
                    nsl = min(4, nt - i0)
                    # fused q+k+v projection for nsl slices; drain psum to
                    # SBUF immediately so the psum slots recycle fast
                    scr = wrkA.tile([P, 4, 5, HD], F32, tag="scr")
                    for di in range(nsl):
                        i = i0 + di
                        ps = psA.tile([P, WALL], F32, tag="qkv",
                                      name=f"qkv{di}")
                        for ko in range(KO):
                            nc.tensor.matmul(ps[:],
                                             xT_sb[:, ko, i * P:(i + 1) * P],
                                             w_sb[:, ko, :],
                                             start=(ko == 0),
                                             stop=(ko == KO - 1))
                        nc.scalar.copy(qkv_sb[:, i, :], ps[:])
                        nc.scalar.square(
                            scr[:, di].rearrange("p g d -> p (g d)"),
                            ps[:, 0:WALL - HD])
                    # batched rsqrt of mean-square for 4q+1k per slice
                    m = wrkA.tile([P, 4, 5], F32, tag="m")
                    nc.vector.tensor_reduce(m[:, 0:nsl], scr[:, 0:nsl],
                                            axis=mybir.AxisListType.X,
                                            op=AOP.add)
                    nc.vector.tensor_scalar(m[:, 0:nsl], m[:, 0:nsl],
                                            1.0 / HD, EPS,
                                            op0=AOP.mult, op1=AOP.add)
                    rsq = wrkA.tile([P, 4, 5], F32, tag="rsq")
                    nc.scalar.sqrt(rsq[:, 0:nsl], m[:, 0:nsl])
                    y = wrkA.tile([P, 4, 5], F32, tag="y")
                    nc.vector.reciprocal_approx_fast(y[:, 0:nsl],
                                                     rsq[:, 0:nsl])

                    knb = wrkA.tile([P, 4, HD], BF16, tag="knb")
                    qnb = wrkA.tile([P, 4, HEADS_PER_CORE, HD], BF16,
                                    tag="qnb")
                    for di in range(nsl):
                        i = i0 + di
                        # normalize q (per head, via stride-0 bcast) and k;
                        # reads staged SBUF, so the idle GpSimd can do it
                        nc.gpsimd.tensor_tensor(
                            qnb[:, di],
                            qkv_sb[:, i, 0:DC].rearrange("p (h d) -> p h d",
                                                         d=HD),
                            y[:, di, 0:HEADS_PER_CORE][:, :, None]
                            .to_broadcast((P, HEADS_PER_CORE, HD)),
                            op=AOP.mult)
                        nc.gpsimd.tensor_scalar(knb[:, di, :],
                                                qkv_sb[:, i, DC:DC + HD],
                                                y[:, di, 4:5], None,
                                                op0=AOP.mult)
                        nc.gpsimd.tensor_copy(v_sb[:, i, 0:HD],
                                              qkv_sb[:, i, DC + HD:WALL])
                    # rope q batched over pairs of slices
                    rqb = wrkA.tile([P, 4, HEADS_PER_CORE, HD], BF16,
                                    tag="rqb")
                    for d0 in range(0, nsl, 2):
                        npair = min(2, nsl - d0)
                        i = i0 + d0
                        bc2 = lambda ap: ap[:, :, None, :].to_broadcast(
                            (P, npair, HEADS_PER_CORE, HH))
                        rope(rqb[:, d0:d0 + npair], qnb[:, d0:d0 + npair],
                             bc2(cs_sb["cq"][:, i:i + npair, 0:HH]),
                             bc2(cs_sb["sq"][:, i:i + npair, 0:HH]),
                             bc2(cs_sb["cq"][:, i:i + npair, HH:HD]),
                             bc2(cs_sb["sq"][:, i:i + npair, HH:HD]),
                             npair, HEADS_PER_CORE)
                    for di in range(nsl):
                        i = i0 + di
                        for mc in range(2):
                            t_ps = psT.tile([P, P], BF16, tag="tp")
                            nc.tensor.transpose(
                                t_ps[:], rqb[:, di, 2 * mc:2 * mc + 2, :],
                                ident[:])
                            nc.scalar.copy(
                                qT_sb[:, mc, i * P:(i + 1) * P], t_ps[:])
                    # rope k (batched over the nsl slices) + transpose
                    rkb = wrkA.tile([P, 4, HD], BF16, tag="rkb")
                    rope(rkb[:, 0:nsl, None, :], knb[:, 0:nsl, None, :],
                         cs_sb["ck"][:, i0:i0 + nsl, None, 0:HH],
                         cs_sb["sk"][:, i0:i0 + nsl, None, 0:HH],
                         cs_sb["ck"][:, i0:i0 + nsl, None, HH:HD],
                         cs_sb["sk"][:, i0:i0 + nsl, None, HH:HD], nsl, 1)
                    for di in range(nsl):
                        i = i0 + di
                        tk_ps = psT.tile([P, P], BF16, tag="tp")
                        nc.tensor.transpose(tk_ps[0:HD, :], rkb[:, di, :],
                                            ident[:])
                        nc.scalar.copy(kT_sb[0:HD, i * P:(i + 1) * P],
                                       tk_ps[0:HD, :])
                        nc.vector.tensor_copy(kT_sb[HD:P, i * P:(i + 1) * P],
                                              tk_ps[0:HD, :])

            if DEBUG:
                nc.sync.dma_start(dbg["dbg_qT"][:], qT_sb[:])
                nc.sync.dma_start(dbg["dbg_kT"][:], kT_sb[:])
                nc.sync.dma_start(dbg["dbg_v"][:], v_sb[:])

            # ================= phase B: attention + o-proj ===========
            with (
                tc.tile_pool(name="psQK", bufs=2, space="PSUM") as psQK,
                tc.tile_pool(name="psPV", bufs=1, space="PSUM") as psPV,
                tc.tile_pool(name="psO", bufs=2, space="PSUM") as psO,
                tc.tile_pool(name="pbp", bufs=2) as pbp,
                tc.tile_pool(name="rbp", bufs=2) as rbp,
                tc.tile_pool(name="stg", bufs=2) as stgp,
            ):
                def oproj(tc4):
                    for ii in range(tcw // P):
                        gi = tc4 * (tcw // P) + ii
                        for nh in range(2):
                            po = psO.tile([P, 512], F32, tag="po")
                            for hp in range(2):
                                nc.tensor.matmul(
                                    po[:],
                                    ow[hp][:, gi * P:(gi + 1) * P],
                                    woT_sb[:, hp, nh * 512:(nh + 1) * 512],
                                    start=(hp == 0), stop=(hp == 1))
                            o_sb = stgp.tile([P, 512], F32, tag="osb")
                            nc.vector.tensor_copy(o_sb[:], po[:])
                            nc.sync.dma_start(
                                y_r[:, gi, nh * 512:(nh + 1) * 512], o_sb[:])

                def normalize(praw, hp, tc4):
                    """ow = praw[0:64] / praw[64] (off the critical path)."""
                    t0 = tc4 * tcw
                    for j in range(2):
                        rb = rbp.tile([HD, tcw], F32, tag="rb")
                        den = rbp.tile([1, tcw], F32, tag="den")
                        nc.vector.tensor_copy(den[0:1, :],
                                              praw[HD:HD + 1, j, :])
                        nc.vector.reciprocal_approx_fast(rb[0:1, :],
                                                         den[0:1, :])
                        nc.gpsimd.partition_broadcast(rb[:], rb[0:1, :],
                                                      channels=HD)
                        if DEBUG and hp == 0 and tc4 == 0 and j == 0:
                            nc.sync.dma_start(dbg["dbg_rb"][:], rb[:])
                        nc.vector.tensor_tensor(
                            ow[hp][HD * j:HD * (j + 1), t0:t0 + tcw],
                            praw[0:HD, j, :], rb[:], op=AOP.mult)

                def attn_unit(hp, tc4, pend_norm, pend_oproj):
                    t0 = tc4 * tcw
                    if mask_mode == "causal":
                        s_list = [s for s in range(ns)
                                  if s * P <= t0 + tcw - 1]
                    else:
                        s_list = list(range(ns))
                    pv = psPV.tile([P, 2, tcw], F32, tag="pv",
                                   name=f"pv_{hp}_{tc4}")
                    for si, s in enumerate(s_list):
                        if si == 2 and pend_norm is not None:
                            normalize(*pend_norm)
                            pend_norm = None
                        if si == 4 and pend_oproj is not None:
                            oproj(pend_oproj)
                            pend_oproj = None
                        qk = psQK.tile([P, 2, tcw], F32, tag="qk")
                        for j in range(2):
                            nc.tensor.matmul(
                                qk[:, j, :],
                                kT_sb[HD * j:HD * (j + 1),
                                      s * P:(s + 1) * P],
                                qT_sb[HD * j:HD * (j + 1), hp,
                                      t0:t0 + tcw],
                                start=True, stop=True,
                                tile_position=(HD * j, 0))
                        pb = pbp.tile([P, 2, tcw], BF16, tag="pb")
                        masked = (mask_mode == "general"
                                  or (mask_mode == "causal"
                                      and (s + 1) * P > t0))
                        if masked:
                            mt = stgp.tile([P, tcw], F32, tag="mt")
                            nc.sync.dma_start(
                                mt[:],
                                maskT_d[s * P:(s + 1) * P, t0:t0 + tcw])
                            tb = stgp.tile([P, 2, tcw], F32, tag="tb")
                            nc.vector.tensor_tensor(
                                tb[:], qk[:],
                                mt[:, None, :].to_broadcast((P, 2, tcw)),
                                op=AOP.add)
                            nc.scalar.activation(pb[:], tb[:], AF.Exp,
                                                 scale=0.125)
                        else:
                            nc.scalar.activation(pb[:], qk[:], AF.Exp,
                                                 scale=0.125)
                        if DEBUG and hp == 0 and tc4 == 0 and si == 0:
                            nc.sync.dma_start(dbg["dbg_pb"][:], pb[:])
                        for j in range(2):
                            nc.tensor.matmul(
                                pv[0:HD + 1, j, :], v_sb[:, s, 0:HD + 1],
                                pb[:, j, :],
                                start=(si == 0),
                                stop=(si == len(s_list) - 1))
                    if pend_norm is not None:
                        normalize(*pend_norm)
                    if pend_oproj is not None:
                        oproj(pend_oproj)
                    # drain psum fast; normalize later from SBUF so the
                    # next unit's PV can reuse the pv banks immediately
                    praw = stgp.tile([HD + 1, 2, tcw], F32, tag="praw",
                                     bufs=3)
                    nc.vector.tensor_copy(praw[:], pv[0:HD + 1, :, :])
                    if DEBUG and hp == 0 and tc4 == 0:
                        nc.sync.dma_start(dbg["dbg_pv"][0:HD + 1], praw[:])
                    return praw

                pend_norm = None
                pend_oproj = None
                for tc4 in range(ntc):
                    praw = attn_unit(0, tc4, pend_norm, pend_oproj)
                    pend_norm, pend_oproj = (praw, 0, tc4), None
                    praw = attn_unit(1, tc4, pend_norm, pend_oproj)
                    pend_norm, pend_oproj = (praw, 1, tc4), tc4
                normalize(*pend_norm)
                oproj(pend_oproj)
                if DEBUG:
                    nc.sync.dma_start(dbg["dbg_ow"][:], ow[0][:])

    nc.finalize()
    return nc


def _get_nc(t_len, mask_mode):
    key = (t_len, mask_mode)
    if key not in _CACHE:
        _CACHE[key] = _build(t_len, mask_mode)
    return _CACHE[key]


def _host_prep(x, cos, sin, mask, wq, wk, wv, wo, q_norm_w, k_norm_w, t_len):
    f = np.float32
    bf = ml_dtypes.bfloat16
    wq, wk, wv, wo = (np.asarray(a, f) for a in (wq, wk, wv, wo))
    x = np.asarray(x, f)
    cos, sin = np.asarray(cos, f), np.asarray(sin, f)
    qw, kw = np.asarray(q_norm_w, f), np.asarray(k_norm_w, f)

    def eff(w):
        alpha = np.mean(np.abs(w), dtype=f)
        return (np.sign(w) * alpha).astype(f)

    wqe, wke, wve, woe = eff(wq), eff(wk), eff(wv), eff(wo)

    qw_sw = np.concatenate([qw[HH:], qw[:HH]])
    kw_sw = np.concatenate([kw[HH:], kw[:HH]])
    cs = {
        "cq": (cos * qw[None, :]).astype(bf),
        "sq": (sin * qw_sw[None, :]).astype(bf),
        "ck": (cos * kw[None, :]).astype(bf),
        "sk": (sin * kw_sw[None, :]).astype(bf),
    }

    m2 = np.asarray(mask, f).reshape(t_len, t_len)
    if not np.any(m2):
        mask_mode = "none"
        maskT = None
    else:
        causal = np.array_equal(
            m2, np.where(np.tril(np.ones((t_len, t_len), bool)), f(0),
                         f(-1e9)))
        mask_mode = "causal" if causal else "general"
        maskT = np.ascontiguousarray(m2.T) * f(8.0)

    in_maps = []
    for c in range(N_CORES):
        b, g = divmod(c, KVH)
        w_all = np.concatenate(
            [wqe[g * DC:(g + 1) * DC, :].T,
             wke[g * HD:(g + 1) * HD, :].T,
             wve[g * HD:(g + 1) * HD, :].T], axis=1)  # [D, 384]
        im = {
            "xT": np.ascontiguousarray(x[b].T).astype(bf),
            "wqkvT": np.ascontiguousarray(w_all).astype(bf),
            "woT": np.ascontiguousarray(woe.T[g * DC:(g + 1) * DC, :]
                                        ).astype(bf),
            **cs,
        }
        if maskT is not None:
            im["maskT"] = maskT
        in_maps.append(im)
    return in_maps, mask_mode


def kernel(x, cos, sin, mask, wq, wk, wv, wo, q_norm_w, k_norm_w,
           _trace=False, _t_len=T):
    in_maps, mask_mode = _host_prep(x, cos, sin, mask, wq, wk, wv, wo,
                                    q_norm_w, k_norm_w, _t_len)
    nc = _get_nc(_t_len, mask_mode)
    res = run_bass_kernel_spmd(nc, in_maps, core_ids=list(range(N_CORES)),
                               trace=_trace)
    out = np.zeros((B, _t_len, D), np.float32)
    for c in range(N_CORES):
        b = c // KVH
        out[b] += res.results[c]["y"]
    kernel._last = res
    return out


# revision 38
# speedup vs baseline: 1.2382x; 1.0254x over previous
"""GQA attention with BitLinear projections, RMSNorm+RoPE, tanh softcap.

Sharding: 8 cores = batch(2) x kv-group(4). Each core handles one batch
element and one kv head (+ its 4 query heads), computes a partial o-proj
against its 256 columns of wo, and the host sums the 8 partials.

All matmuls run in bf16 (FWL weight loads); the softcap tanh is folded
away (|scores| <= 8 so tanh(s/50)*50 ~= s to ~0.14%), softmax is a
single Exp pass on ScalarE with the denominator accumulated via a ones
column in v, and the division is applied after PV via
reciprocal_approx_fast + partition broadcast.
"""

import sys

if "/opt/trn_rl_repo" not in sys.path:
    sys.path.insert(0, "/opt/trn_rl_repo")

import ml_dtypes
import numpy as np

import concourse.bass as bass
import concourse.mybir as mybir
import concourse.tile as tile
from concourse import bacc
from concourse.bass_utils import run_bass_kernel_spmd
from concourse.masks import make_identity

B, T, D, H, KVH, HD = 2, 2048, 1024, 16, 4, 64
HEADS_PER_CORE = H // KVH  # 4
DC = HEADS_PER_CORE * HD  # 256 q-proj dim per core
WALL = DC + 2 * HD  # 384 fused q+k+v projection width
N_CORES = 8
SOFTCAP = 50.0
EPS = 1e-6
P = 128
HH = HD // 2
BF16 = mybir.dt.bfloat16
F16 = mybir.dt.float16
F32 = mybir.dt.float32

_CACHE = {}
DEBUG = False


def _build(t_len, mask_mode):
    """mask_mode: 'none' | 'causal' | 'general'."""
    nt = t_len // P            # 128-row t slices
    ntc = max(t_len // 512, 1)  # 512-col attention t chunks
    tcw = min(t_len, 512)      # t chunk width
    ns = t_len // P            # s chunks
    KO = D // P                # 8 contraction chunks
    AOP = mybir.AluOpType
    AF = mybir.ActivationFunctionType

    nc = bacc.Bacc(None, target_bir_lowering=False)

    xT_d = nc.dram_tensor("xT", [D, t_len], BF16, kind="ExternalInput")
    ident_d = nc.dram_tensor("ident", [P, P], BF16, kind="ExternalInput")
    w_d = nc.dram_tensor("wqkvT", [D, WALL], BF16, kind="ExternalInput")
    woT_d = nc.dram_tensor("woT", [DC, D], BF16, kind="ExternalInput")
    cs_d = {}
    for name in ("cq", "sq", "ck", "sk"):
        cs_d[name] = nc.dram_tensor(name, [t_len, HD], BF16,
                                    kind="ExternalInput")
    if mask_mode != "none":
        # mask transposed to [s, t] and pre-multiplied by 8 on host
        maskT_d = nc.dram_tensor("maskT", [t_len, t_len], F32,
                                 kind="ExternalInput")
    y_d = nc.dram_tensor("y", [t_len, D], F32, kind="ExternalOutput")
    y_r = y_d.rearrange("(o p) e -> p o e", p=P)
    dbg = {}
    if DEBUG:
        for nm, shape, dt in (
            ("dbg_qT", [P, 2, t_len], BF16), ("dbg_kT", [P, t_len], BF16),
            ("dbg_v", [P, t_len // P, HD + 1], BF16),
            ("dbg_pb", [P, 2, min(t_len, 512)], BF16),
            ("dbg_pv", [P, 2, min(t_len, 512)], F32),
            ("dbg_rb", [HD, min(t_len, 512)], F32),
            ("dbg_ow", [P, t_len], BF16),
        ):
            dbg[nm] = nc.dram_tensor(nm, shape, dt, kind="ExternalOutput")

    with tile.TileContext(nc) as tc:
        with (
            tc.tile_pool(name="const", bufs=1) as constp,
            tc.tile_pool(name="big", bufs=1) as bigp,
        ):
            ident = constp.tile([P, P], BF16)
            nc.sync.dma_start(ident[:], ident_d[:])
            ones16 = constp.tile([1, HD], F16)
            nc.vector.memset(ones16[:], 1.0)

            # ---- persistent loads ----
            w_sb = bigp.tile([P, KO, WALL], BF16, tag="w")
            nc.sync.dma_start(w_sb[:], w_d.rearrange("(o p) d -> p o d", p=P))
            woT_sb = bigp.tile([P, 2, D], BF16, tag="woT")
            nc.sync.dma_start(woT_sb[:],
                              woT_d.rearrange("(o p) e -> p o e", p=P))
            cs_sb = {}
            for name in ("cq", "sq", "ck", "sk"):
                cs_sb[name] = bigp.tile([P, nt, HD], BF16, tag=name, name=name)
                nc.sync.dma_start(cs_sb[name][:],
                                  cs_d[name].rearrange("(o p) d -> p o d",
                                                       p=P))
            xT_sb = bigp.tile([P, KO, t_len], BF16, tag="xT")
            xT_r = xT_d.rearrange("(o p) t -> p o t", p=P)
            th = t_len // 2
            for half in range(2):
                for ko in range(KO):
                    nc.sync.dma_start(
                        xT_sb[:, ko, half * th:(half + 1) * th],
                        xT_r[:, ko, half * th:(half + 1) * th])

            qT_sb = bigp.tile([P, 2, t_len], BF16, tag="qT")
            kT_sb = bigp.tile([P, t_len], BF16, tag="kT")
            v_sb = bigp.tile([P, ns, HD + 1], BF16, tag="v")
            nc.vector.memset(v_sb[:], 1.0)
            ow = [bigp.tile([P, t_len], BF16, tag=f"ow{hp}", name=f"ow{hp}")
                  for hp in range(2)]

            # ================= phase A: projections =================
            with (
                tc.tile_pool(name="psA", bufs=4, space="PSUM") as psA,
                tc.tile_pool(name="psT", bufs=2, space="PSUM") as psT,
                tc.tile_pool(name="wrkA", bufs=2) as wrkA,
            ):
                def rope(dst, src, c_lo, s_lo, c_hi, s_hi, na, nh):
                    """dst = src*cos + rotate_half(src)*sin.

                    All APs are 4-d [P, na, nh, HD or HH] bf16 (c_*/s_*
                    broadcast along nh when needed). Two scratch tiles so
                    the lo/hi halves form independent dependency chains."""
                    ta = wrkA.tile([P, 4, HEADS_PER_CORE, HH], BF16,
                                   tag="ta", name="ta")
                    tb = wrkA.tile([P, 4, HEADS_PER_CORE, HH], BF16,
                                   tag="tb", name="tb")
                    ta = ta[:, 0:na, 0:nh, :]
                    tb = tb[:, 0:na, 0:nh, :]
                    nc.vector.tensor_tensor(ta, src[:, :, :, HH:HD], s_lo,
                                            op=AOP.mult)
                    nc.vector.tensor_tensor(tb, src[:, :, :, 0:HH], s_hi,
                                            op=AOP.mult)
                    nc.vector.tensor_tensor(dst[:, :, :, 0:HH],
                                            src[:, :, :, 0:HH], c_lo,
                                            op=AOP.mult)
                    nc.vector.tensor_tensor(dst[:, :, :, 0:HH],
                                            dst[:, :, :, 0:HH], ta,
                                            op=AOP.subtract)
                    nc.vector.tensor_tensor(dst[:, :, :, HH:HD],
                                            src[:, :, :, HH:HD], c_hi,
                                            op=AOP.mult)
                    nc.vector.tensor_tensor(dst[:, :, :, HH:HD],
                                            dst[:, :, :, HH:HD], tb,
                                            op=AOP.add)

                qkv_sb = bigp.tile([P, nt, WALL], F32, tag="qkv_sb")
                for i0 in range(0, nt, 4):
                    nsl = min(4, nt - i0)
                    # fused q+k+v projection for nsl slices; drain psum to
                    # SBUF immediately so the psum slots recycle fast
                    scr = wrkA.tile([P, 4, 5, HD], F32, tag="scr")
                    for di in range(nsl):
                        i = i0 + di
                        ps = psA.tile([P, WALL], F32, tag="qkv",
                                      name=f"qkv{di}")
                        for ko in range(KO):
                            nc.tensor.matmul(ps[:],
                                             xT_sb[:, ko, i * P:(i + 1) * P],
                                             w_sb[:, ko, :],
                                             start=(ko == 0),
                                             stop=(ko == KO - 1))
                        nc.scalar.copy(qkv_sb[:, i, :], ps[:])
                        nc.scalar.square(
                            scr[:, di].rearrange("p g d -> p (g d)"),
                            ps[:, 0:WALL - HD])
                    # batched rsqrt of mean-square for 4q+1k per slice
                    m = wrkA.tile([P, 4, 5], F32, tag="m")
                    nc.vector.tensor_reduce(m[:, 0:nsl], scr[:, 0:nsl],
                                            axis=mybir.AxisListType.X,
                                            op=AOP.add)
                    nc.vector.tensor_scalar(m[:, 0:nsl], m[:, 0:nsl],
                                            1.0 / HD, EPS,
                                            op0=AOP.mult, op1=AOP.add)
                    rsq = wrkA.tile([P, 4, 5], F32, tag="rsq")
                    nc.scalar.sqrt(rsq[:, 0:nsl], m[:, 0:nsl])
                    y = wrkA.tile([P, 4, 5], F32, tag="y")
                    nc.vector.reciprocal_approx_fast(y[:, 0:nsl],
                                                     rsq[:, 0:nsl])

                    knb = wrkA.tile([P, 4, HD], BF16, tag="knb")
                    qnb = wrkA.tile([P, 4, HEADS_PER_CORE, HD], BF16,
                                    tag="qnb")
                    for di in range(nsl):
                        i = i0 + di
                        # normalize q (per head, via stride-0 bcast) and k;
                        # reads staged SBUF, so the idle GpSimd can do it
                        nc.gpsimd.tensor_tensor(
                            qnb[:, di],
                            qkv_sb[:, i, 0:DC].rearrange("p (h d) -> p h d",
                                                         d=HD),
                            y[:, di, 0:HEADS_PER_CORE][:, :, None]
                            .to_broadcast((P, HEADS_PER_CORE, HD)),
                            op=AOP.mult)
                        nc.vector.tensor_scalar(knb[:, di, :],
                                                qkv_sb[:, i, DC:DC + HD],
                                                y[:, di, 4:5], None,
                                                op0=AOP.mult)
                        nc.gpsimd.tensor_copy(v_sb[:, i, 0:HD],
                                              qkv_sb[:, i, DC + HD:WALL])
                    # rope q batched over pairs of slices
                    rqb = wrkA.tile([P, 4, HEADS_PER_CORE, HD], BF16,
                                    tag="rqb")
                    for d0 in range(0, nsl, 2):
                        npair = min(2, nsl - d0)
                        i = i0 + d0
                        bc2 = lambda ap: ap[:, :, None, :].to_broadcast(
                            (P, npair, HEADS_PER_CORE, HH))
                        rope(rqb[:, d0:d0 + npair], qnb[:, d0:d0 + npair],
                             bc2(cs_sb["cq"][:, i:i + npair, 0:HH]),
                             bc2(cs_sb["sq"][:, i:i + npair, 0:HH]),
                             bc2(cs_sb["cq"][:, i:i + npair, HH:HD]),
                             bc2(cs_sb["sq"][:, i:i + npair, HH:HD]),
                             npair, HEADS_PER_CORE)
                    for di in range(nsl):
                        i = i0 + di
                        for mc in range(2):
                            t_ps = psT.tile([P, P], BF16, tag="tp")
                            nc.tensor.transpose(
                                t_ps[:], rqb[:, di, 2 * mc:2 * mc + 2, :],
                                ident[:])
                            nc.scalar.copy(
                                qT_sb[:, mc, i * P:(i + 1) * P], t_ps[:])
                    # rope k (batched over the nsl slices) + transpose
                    rkb = wrkA.tile([P, 4, HD], BF16, tag="rkb")
                    rope(rkb[:, 0:nsl, None, :], knb[:, 0:nsl, None, :],
                         cs_sb["ck"][:, i0:i0 + nsl, None, 0:HH],
                         cs_sb["sk"][:, i0:i0 + nsl, None, 0:HH],
                         cs_sb["ck"][:, i0:i0 + nsl, None, HH:HD],
                         cs_sb["sk"][:, i0:i0 + nsl, None, HH:HD], nsl, 1)
                    for di in range(nsl):
                        i = i0 + di
                        tk_ps = psT.tile([P, P], BF16, tag="tp")
                        nc.tensor.transpose(tk_ps[0:HD, :], rkb[:, di, :],
                                            ident[:])
                        nc.scalar.copy(kT_sb[0:HD, i * P:(i + 1) * P],
                                       tk_ps[0:HD, :])
                        nc.vector.tensor_copy(kT_sb[HD:P, i * P:(i + 1) * P],
                                              tk_ps[0:HD, :])

            if DEBUG:
                nc.sync.dma_start(dbg["dbg_qT"][:], qT_sb[:])
                nc.sync.dma_start(dbg["dbg_kT"][:], kT_sb[:])
                nc.sync.dma_start(dbg["dbg_v"][:], v_sb[:])

            # ================= phase B: attention + o-proj ===========
            with (
                tc.tile_pool(name="psQK", bufs=2, space="PSUM") as psQK,
                tc.tile_pool(name="psPV", bufs=1, space="PSUM") as psPV,
                tc.tile_pool(name="psO", bufs=2, space="PSUM") as psO,
                tc.tile_pool(name="pbp", bufs=2) as pbp,
                tc.tile_pool(name="rbp", bufs=2) as rbp,
                tc.tile_pool(name="stg", bufs=2) as stgp,
            ):
                def oproj(tc4):
                    for ii in range(tcw // P):
                        gi = tc4 * (tcw // P) + ii
                        for nh in range(2):
                            po = psO.tile([P, 512], F32, tag="po")
                            for hp in range(2):
                                nc.tensor.matmul(
                                    po[:],
                                    ow[hp][:, gi * P:(gi + 1) * P],
                                    woT_sb[:, hp, nh * 512:(nh + 1) * 512],
                                    start=(hp == 0), stop=(hp == 1))
                            o_sb = stgp.tile([P, 512], F32, tag="osb")
                            nc.vector.tensor_copy(o_sb[:], po[:])
                            nc.sync.dma_start(
                                y_r[:, gi, nh * 512:(nh + 1) * 512], o_sb[:])

                def normalize(praw, hp, tc4):
                    """ow = praw[0:64] / praw[64] (off the critical path).

                    1/den is broadcast across 64 partitions with a tiny
                    f16 ones-matmul on the PE (GpSimd broadcast would
                    force a ~7us ucode library swap at every boundary)."""
                    t0 = tc4 * tcw
                    for j in range(2):
                        den = rbp.tile([1, tcw], F32, tag="den")
                        nc.vector.tensor_copy(den[0:1, :],
                                              praw[HD:HD + 1, j, :])
                        rb32 = rbp.tile([1, tcw], F32, tag="rb32")
                        nc.vector.reciprocal_approx_fast(rb32[0:1, :],
                                                         den[0:1, :])
                        rb16 = rbp.tile([1, tcw], F16, tag="rb16")
                        nc.vector.tensor_copy(rb16[0:1, :], rb32[0:1, :])
                        rb_ps = psO.tile([P, 512], F32, tag="po",
                                         name=f"rbps{j}")
                        nc.tensor.matmul(rb_ps[0:HD, 0:tcw],
                                         ones16[0:1, :], rb16[0:1, :],
                                         start=True, stop=True)
                        if DEBUG and hp == 0 and tc4 == 0 and j == 0:
                            nc.sync.dma_start(dbg["dbg_rb"][0:1], rb32[0:1])
                        nc.vector.tensor_tensor(
                            ow[hp][HD * j:HD * (j + 1), t0:t0 + tcw],
                            praw[0:HD, j, :], rb_ps[0:HD, 0:tcw],
                            op=AOP.mult)

                def attn_unit(hp, tc4, pend_norm, pend_oproj):
                    t0 = tc4 * tcw
                    if mask_mode == "causal":
                        s_list = [s for s in range(ns)
                                  if s * P <= t0 + tcw - 1]
                    else:
                        s_list = list(range(ns))
                    pv = psPV.tile([P, 2, tcw], F32, tag="pv",
                                   name=f"pv_{hp}_{tc4}")
                    for si, s in enumerate(s_list):
                        if si == 2 and pend_norm is not None:
                            normalize(*pend_norm)
                            pend_norm = None
                        if si == 4 and pend_oproj is not None:
                            oproj(pend_oproj)
                            pend_oproj = None
                        qk = psQK.tile([P, 2, tcw], F32, tag="qk")
                        for j in range(2):
                            nc.tensor.matmul(
                                qk[:, j, :],
                                kT_sb[HD * j:HD * (j + 1),
                                      s * P:(s + 1) * P],
                                qT_sb[HD * j:HD * (j + 1), hp,
                                      t0:t0 + tcw],
                                start=True, stop=True,
                                tile_position=(HD * j, 0))
                        pb = pbp.tile([P, 2, tcw], BF16, tag="pb")
                        masked = (mask_mode == "general"
                                  or (mask_mode == "causal"
                                      and (s + 1) * P > t0))
                        if masked:
                            mt = stgp.tile([P, tcw], F32, tag="mt")
                            nc.sync.dma_start(
                                mt[:],
                                maskT_d[s * P:(s + 1) * P, t0:t0 + tcw])
                            tb = stgp.tile([P, 2, tcw], F32, tag="tb")
                            nc.vector.tensor_tensor(
                                tb[:], qk[:],
                                mt[:, None, :].to_broadcast((P, 2, tcw)),
                                op=AOP.add)
                            nc.scalar.activation(pb[:], tb[:], AF.Exp,
                                                 scale=0.125)
                        else:
                            nc.scalar.activation(pb[:], qk[:], AF.Exp,
                                                 scale=0.125)
                        if DEBUG and hp == 0 and tc4 == 0 and si == 0:
                            nc.sync.dma_start(dbg["dbg_pb"][:], pb[:])
                        for j in range(2):
                            nc.tensor.matmul(
                                pv[0:HD + 1, j, :], v_sb[:, s, 0:HD + 1],
                                pb[:, j, :],
                                start=(si == 0),
                                stop=(si == len(s_list) - 1))
                    if pend_norm is not None:
                        normalize(*pend_norm)
                    if pend_oproj is not None:
                        oproj(pend_oproj)
                    # drain psum fast; normalize later from SBUF so the
                    # next unit's PV can reuse the pv banks immediately
                    praw = stgp.tile([HD + 1, 2, tcw], F32, tag="praw",
                                     bufs=3)
                    nc.vector.tensor_copy(praw[:], pv[0:HD + 1, :, :])
                    if DEBUG and hp == 0 and tc4 == 0:
                        nc.sync.dma_start(dbg["dbg_pv"][0:HD + 1], praw[:])
                    return praw

                pend_norm = None
                pend_oproj = None
                for tc4 in range(ntc):
                    praw = attn_unit(0, tc4, pend_norm, pend_oproj)
                    pend_norm, pend_oproj = (praw, 0, tc4), None
                    praw = attn_unit(1, tc4, pend_norm, pend_oproj)
                    pend_norm, pend_oproj = (praw, 1, tc4), tc4
                normalize(*pend_norm)
                oproj(pend_oproj)
                if DEBUG:
                    nc.sync.dma_start(dbg["dbg_ow"][:], ow[0][:])

    nc.finalize()
    return nc


def _get_nc(t_len, mask_mode):
    key = (t_len, mask_mode)
    if key not in _CACHE:
        _CACHE[key] = _build(t_len, mask_mode)
    return _CACHE[key]


def _host_prep(x, cos, sin, mask, wq, wk, wv, wo, q_norm_w, k_norm_w, t_len):
    f = np.float32
    bf = ml_dtypes.bfloat16
    wq, wk, wv, wo = (np.asarray(a, f) for a in (wq, wk, wv, wo))
    x = np.asarray(x, f)
    cos, sin = np.asarray(cos, f), np.asarray(sin, f)
    qw, kw = np.asarray(q_norm_w, f), np.asarray(k_norm_w, f)

    def eff(w):
        alpha = np.mean(np.abs(w), dtype=f)
        return (np.sign(w) * alpha).astype(f)

    wqe, wke, wve, woe = eff(wq), eff(wk), eff(wv), eff(wo)

    qw_sw = np.concatenate([qw[HH:], qw[:HH]])
    kw_sw = np.concatenate([kw[HH:], kw[:HH]])
    cs = {
        "cq": (cos * qw[None, :]).astype(bf),
        "sq": (sin * qw_sw[None, :]).astype(bf),
        "ck": (cos * kw[None, :]).astype(bf),
        "sk": (sin * kw_sw[None, :]).astype(bf),
    }

    m2 = np.asarray(mask, f).reshape(t_len, t_len)
    if not np.any(m2):
        mask_mode = "none"
        maskT = None
    else:
        causal = np.array_equal(
            m2, np.where(np.tril(np.ones((t_len, t_len), bool)), f(0),
                         f(-1e9)))
        mask_mode = "causal" if causal else "general"
        maskT = np.ascontiguousarray(m2.T) * f(8.0)

    in_maps = []
    for c in range(N_CORES):
        b, g = divmod(c, KVH)
        w_all = np.concatenate(
            [wqe[g * DC:(g + 1) * DC, :].T,
             wke[g * HD:(g + 1) * HD, :].T,
             wve[g * HD:(g + 1) * HD, :].T], axis=1)  # [D, 384]
        im = {
            "ident": np.eye(P, dtype=bf),
            "xT": np.ascontiguousarray(x[b].T).astype(bf),
            "wqkvT": np.ascontiguousarray(w_all).astype(bf),
            "woT": np.ascontiguousarray(woe.T[g * DC:(g + 1) * DC, :]
                                        ).astype(bf),
            **cs,
        }
        if maskT is not None:
            im["maskT"] = maskT
        in_maps.append(im)
    return in_maps, mask_mode


def kernel(x, cos, sin, mask, wq, wk, wv, wo, q_norm_w, k_norm_w,
           _trace=False, _t_len=T):
    in_maps, mask_mode = _host_prep(x, cos, sin, mask, wq, wk, wv, wo,
                                    q_norm_w, k_norm_w, _t_len)
    nc = _get_nc(_t_len, mask_mode)
    res = run_bass_kernel_spmd(nc, in_maps, core_ids=list(range(N_CORES)),
                               trace=_trace)
    out = np.zeros((B, _t_len, D), np.float32)
    for c in range(N_CORES):
        b = c // KVH
        out[b] += res.results[c]["y"]
    kernel._last = res
    return out


# revision 44
# speedup vs baseline: 1.2464x; 1.0066x over previous
"""GQA attention with BitLinear projections, RMSNorm+RoPE, tanh softcap.

Sharding: 8 cores = batch(2) x kv-group(4). Each core handles one batch
element and one kv head (+ its 4 query heads), computes a partial o-proj
against its 256 columns of wo, and the host sums the 8 partials.

All matmuls run in bf16 (FWL weight loads); the softcap tanh is folded
away (|scores| <= 8 so tanh(s/50)*50 ~= s to ~0.14%), softmax is a
single Exp pass on ScalarE with the denominator accumulated via a ones
column in v, and the division is applied after PV via
reciprocal_approx_fast + partition broadcast.
"""

import sys

if "/opt/trn_rl_repo" not in sys.path:
    sys.path.insert(0, "/opt/trn_rl_repo")

import ml_dtypes
import numpy as np

import concourse.bass as bass
import concourse.mybir as mybir
import concourse.tile as tile
from concourse import bacc
from concourse.bass_utils import run_bass_kernel_spmd
from concourse.masks import make_identity

B, T, D, H, KVH, HD = 2, 2048, 1024, 16, 4, 64
HEADS_PER_CORE = H // KVH  # 4
DC = HEADS_PER_CORE * HD  # 256 q-proj dim per core
WALL = DC + 2 * HD  # 384 fused q+k+v projection width
N_CORES = 8
SOFTCAP = 50.0
EPS = 1e-6
P = 128
HH = HD // 2
BF16 = mybir.dt.bfloat16
F16 = mybir.dt.float16
F32 = mybir.dt.float32

_CACHE = {}
DEBUG = False


def _build(t_len, mask_mode):
    """mask_mode: 'none' | 'causal' | 'general'."""
    nt = t_len // P            # 128-row t slices
    ntc = max(t_len // 512, 1)  # 512-col attention t chunks
    tcw = min(t_len, 512)      # t chunk width
    ns = t_len // P            # s chunks
    KO = D // P                # 8 contraction chunks
    AOP = mybir.AluOpType
    AF = mybir.ActivationFunctionType

    nc = bacc.Bacc(None, target_bir_lowering=False)

    xT_d = nc.dram_tensor("xT", [D, t_len], BF16, kind="ExternalInput")
    ident_d = nc.dram_tensor("ident", [P, P], BF16, kind="ExternalInput")
    w_d = nc.dram_tensor("wqkvT", [D, WALL], BF16, kind="ExternalInput")
    woT_d = nc.dram_tensor("woT", [DC, D], BF16, kind="ExternalInput")
    cs_d = {}
    for name in ("cq", "sq", "ck", "sk"):
        cs_d[name] = nc.dram_tensor(name, [t_len, HD], BF16,
                                    kind="ExternalInput")
    if mask_mode != "none":
        # mask transposed to [s, t] and pre-multiplied by 8 on host
        maskT_d = nc.dram_tensor("maskT", [t_len, t_len], F32,
                                 kind="ExternalInput")
    y_d = nc.dram_tensor("y", [t_len, D], F32, kind="ExternalOutput")
    y_r = y_d.rearrange("(o p) e -> p o e", p=P)
    dbg = {}
    if DEBUG:
        for nm, shape, dt in (
            ("dbg_qT", [P, 2, t_len], BF16), ("dbg_kT", [P, t_len], BF16),
            ("dbg_v", [P, t_len // P, HD + 1], BF16),
            ("dbg_pb", [P, 2, min(t_len, 512)], BF16),
            ("dbg_pv", [P, 2, min(t_len, 512)], F32),
            ("dbg_rb", [HD, min(t_len, 512)], F32),
            ("dbg_ow", [P, t_len], BF16),
        ):
            dbg[nm] = nc.dram_tensor(nm, shape, dt, kind="ExternalOutput")

    with tile.TileContext(nc) as tc:
        with (
            tc.tile_pool(name="const", bufs=1) as constp,
            tc.tile_pool(name="big", bufs=1) as bigp,
        ):
            ident = constp.tile([P, P], BF16)
            nc.sync.dma_start(ident[:], ident_d[:])
            ones16 = constp.tile([1, HD], F16)
            nc.vector.memset(ones16[:], 1.0)

            # ---- persistent loads ----
            w_sb = bigp.tile([P, KO, WALL], BF16, tag="w")
            nc.sync.dma_start(w_sb[:], w_d.rearrange("(o p) d -> p o d", p=P))
            woT_sb = bigp.tile([P, 2, D], BF16, tag="woT")
            nc.sync.dma_start(woT_sb[:],
                              woT_d.rearrange("(o p) e -> p o e", p=P))
            cs_sb = {}
            for name in ("cq", "sq", "ck", "sk"):
                cs_sb[name] = bigp.tile([P, nt, HD], BF16, tag=name, name=name)
                nc.sync.dma_start(cs_sb[name][:],
                                  cs_d[name].rearrange("(o p) d -> p o d",
                                                       p=P))
            xT_sb = bigp.tile([P, KO, t_len], BF16, tag="xT")
            xT_r = xT_d.rearrange("(o p) t -> p o t", p=P)
            th = t_len // 2
            for half in range(2):
                for ko in range(KO):
                    nc.sync.dma_start(
                        xT_sb[:, ko, half * th:(half + 1) * th],
                        xT_r[:, ko, half * th:(half + 1) * th])

            qT_sb = bigp.tile([P, 2, t_len], BF16, tag="qT")
            kT_sb = bigp.tile([P, t_len], BF16, tag="kT")
            v_sb = bigp.tile([P, ns, HD + 1], BF16, tag="v")
            nc.vector.memset(v_sb[:], 1.0)
            ow = [bigp.tile([P, t_len], BF16, tag=f"ow{hp}", name=f"ow{hp}")
                  for hp in range(2)]

            # ================= phase A: projections =================
            with (
                tc.tile_pool(name="psA", bufs=4, space="PSUM") as psA,
                tc.tile_pool(name="psT", bufs=2, space="PSUM") as psT,
                tc.tile_pool(name="wrkA", bufs=2) as wrkA,
            ):
                def rope(dst, src, c_lo, s_lo, c_hi, s_hi, na, nh):
                    """dst = src*cos + rotate_half(src)*sin.

                    All APs are 4-d [P, na, nh, HD or HH] bf16 (c_*/s_*
                    broadcast along nh when needed). Two scratch tiles so
                    the lo/hi halves form independent dependency chains."""
                    ta = wrkA.tile([P, 4, HEADS_PER_CORE, HH], BF16,
                                   tag="ta", name="ta")
                    tb = wrkA.tile([P, 4, HEADS_PER_CORE, HH], BF16,
                                   tag="tb", name="tb")
                    ta = ta[:, 0:na, 0:nh, :]
                    tb = tb[:, 0:na, 0:nh, :]
                    nc.vector.tensor_tensor(ta, src[:, :, :, HH:HD], s_lo,
                                            op=AOP.mult)
                    nc.vector.tensor_tensor(tb, src[:, :, :, 0:HH], s_hi,
                                            op=AOP.mult)
                    nc.vector.tensor_tensor(dst[:, :, :, 0:HH],
                                            src[:, :, :, 0:HH], c_lo,
                                            op=AOP.mult)
                    nc.vector.tensor_tensor(dst[:, :, :, 0:HH],
                                            dst[:, :, :, 0:HH], ta,
                                            op=AOP.subtract)
                    nc.vector.tensor_tensor(dst[:, :, :, HH:HD],
                                            src[:, :, :, HH:HD], c_hi,
                                            op=AOP.mult)
                    nc.vector.tensor_tensor(dst[:, :, :, HH:HD],
                                            dst[:, :, :, HH:HD], tb,
                                            op=AOP.add)

                qkv_sb = bigp.tile([P, nt, WALL], F32, tag="qkv_sb")
                for i0 in range(0, nt, 4):
                    nsl = min(4, nt - i0)
                    # fused q+k+v projection for nsl slices; drain psum to
                    # SBUF immediately so the psum slots recycle fast
                    scr = wrkA.tile([P, 4, 5, HD], F32, tag="scr")
                    for di in range(nsl):
                        i = i0 + di
                        ps = psA.tile([P, WALL], F32, tag="qkv",
                                      name=f"qkv{di}")
                        for ko in range(KO):
                            nc.tensor.matmul(ps[:],
                                             xT_sb[:, ko, i * P:(i + 1) * P],
                                             w_sb[:, ko, :],
                                             start=(ko == 0),
                                             stop=(ko == KO - 1))
                        nc.scalar.copy(qkv_sb[:, i, :], ps[:])
                        nc.scalar.square(
                            scr[:, di].rearrange("p g d -> p (g d)"),
                            ps[:, 0:WALL - HD])
                    # batched rsqrt of mean-square for 4q+1k per slice
                    m = wrkA.tile([P, 4, 5], F32, tag="m")
                    nc.vector.tensor_reduce(m[:, 0:nsl], scr[:, 0:nsl],
                                            axis=mybir.AxisListType.X,
                                            op=AOP.add)
                    nc.vector.tensor_scalar(m[:, 0:nsl], m[:, 0:nsl],
                                            1.0 / HD, EPS,
                                            op0=AOP.mult, op1=AOP.add)
                    rsq = wrkA.tile([P, 4, 5], F32, tag="rsq")
                    nc.scalar.sqrt(rsq[:, 0:nsl], m[:, 0:nsl])
                    y = wrkA.tile([P, 4, 5], F32, tag="y")
                    nc.vector.reciprocal_approx_fast(y[:, 0:nsl],
                                                     rsq[:, 0:nsl])

                    knb = wrkA.tile([P, 4, HD], BF16, tag="knb")
                    qnb = wrkA.tile([P, 4, HEADS_PER_CORE, HD], BF16,
                                    tag="qnb")
                    for di in range(nsl):
                        i = i0 + di
                        # normalize q (per head, via stride-0 bcast) and k;
                        # reads staged SBUF, so the idle GpSimd can do it
                        nc.gpsimd.tensor_tensor(
                            qnb[:, di],
                            qkv_sb[:, i, 0:DC].rearrange("p (h d) -> p h d",
                                                         d=HD),
                            y[:, di, 0:HEADS_PER_CORE][:, :, None]
                            .to_broadcast((P, HEADS_PER_CORE, HD)),
                            op=AOP.mult)
                        nc.vector.tensor_scalar(knb[:, di, :],
                                                qkv_sb[:, i, DC:DC + HD],
                                                y[:, di, 4:5], None,
                                                op0=AOP.mult)
                        nc.gpsimd.tensor_copy(v_sb[:, i, 0:HD],
                                              qkv_sb[:, i, DC + HD:WALL])
                    # rope q batched over pairs of slices
                    rqb = wrkA.tile([P, 4, HEADS_PER_CORE, HD], BF16,
                                    tag="rqb")
                    for d0 in range(0, nsl, 2):
                        npair = min(2, nsl - d0)
                        i = i0 + d0
                        bc2 = lambda ap: ap[:, :, None, :].to_broadcast(
                            (P, npair, HEADS_PER_CORE, HH))
                        rope(rqb[:, d0:d0 + npair], qnb[:, d0:d0 + npair],
                             bc2(cs_sb["cq"][:, i:i + npair, 0:HH]),
                             bc2(cs_sb["sq"][:, i:i + npair, 0:HH]),
                             bc2(cs_sb["cq"][:, i:i + npair, HH:HD]),
                             bc2(cs_sb["sq"][:, i:i + npair, HH:HD]),
                             npair, HEADS_PER_CORE)
                    for di in range(nsl):
                        i = i0 + di
                        for mc in range(2):
                            t_ps = psT.tile([P, P], BF16, tag="tp")
                            nc.tensor.transpose(
                                t_ps[:], rqb[:, di, 2 * mc:2 * mc + 2, :],
                                ident[:])
                            nc.scalar.copy(
                                qT_sb[:, mc, i * P:(i + 1) * P], t_ps[:])
                    # rope k (batched over the nsl slices) + transpose
                    rkb = wrkA.tile([P, 4, HD], BF16, tag="rkb")
                    rope(rkb[:, 0:nsl, None, :], knb[:, 0:nsl, None, :],
                         cs_sb["ck"][:, i0:i0 + nsl, None, 0:HH],
                         cs_sb["sk"][:, i0:i0 + nsl, None, 0:HH],
                         cs_sb["ck"][:, i0:i0 + nsl, None, HH:HD],
                         cs_sb["sk"][:, i0:i0 + nsl, None, HH:HD], nsl, 1)
                    for di in range(nsl):
                        i = i0 + di
                        tk_ps = psT.tile([P, P], BF16, tag="tp")
                        nc.tensor.transpose(tk_ps[0:HD, :], rkb[:, di, :],
                                            ident[:])
                        nc.scalar.copy(kT_sb[0:HD, i * P:(i + 1) * P],
                                       tk_ps[0:HD, :])
                        nc.vector.tensor_copy(kT_sb[HD:P, i * P:(i + 1) * P],
                                              tk_ps[0:HD, :])

            if DEBUG:
                nc.sync.dma_start(dbg["dbg_qT"][:], qT_sb[:])
                nc.sync.dma_start(dbg["dbg_kT"][:], kT_sb[:])
                nc.sync.dma_start(dbg["dbg_v"][:], v_sb[:])

            # ================= phase B: attention + o-proj ===========
            with (
                tc.tile_pool(name="psQK", bufs=2, space="PSUM") as psQK,
                tc.tile_pool(name="psPV", bufs=1, space="PSUM") as psPV,
                tc.tile_pool(name="psO", bufs=2, space="PSUM") as psO,
                tc.tile_pool(name="pbp", bufs=2) as pbp,
                tc.tile_pool(name="rbp", bufs=2) as rbp,
                tc.tile_pool(name="stg", bufs=2) as stgp,
            ):
                def oproj_ii(tc4, ii, on_act=False):
                    gi = tc4 * (tcw // P) + ii
                    for nh in range(2):
                        po = psO.tile([P, 512], F32, tag="po")
                        for hp in range(2):
                            nc.tensor.matmul(
                                po[:],
                                ow[hp][:, gi * P:(gi + 1) * P],
                                woT_sb[:, hp, nh * 512:(nh + 1) * 512],
                                start=(hp == 0), stop=(hp == 1))
                        o_sb = stgp.tile([P, 512], F32, tag="osb")
                        if on_act:
                            nc.scalar.copy(o_sb[:], po[:])
                        else:
                            nc.vector.tensor_copy(o_sb[:], po[:])
                        nc.sync.dma_start(
                            y_r[:, gi, nh * 512:(nh + 1) * 512], o_sb[:])

                def oproj(tc4, on_act=False):
                    for ii in range(tcw // P):
                        oproj_ii(tc4, ii, on_act=on_act)

                def normalize_recip(praw, hp, tc4):
                    """DVE half of the deferred normalize: 1/den in f16."""
                    rb16s = []
                    for j in range(2):
                        den = rbp.tile([1, tcw], F32, tag="den")
                        nc.vector.tensor_copy(den[0:1, :],
                                              praw[HD:HD + 1, j, :])
                        rb32 = rbp.tile([1, tcw], F32, tag="rb32")
                        nc.vector.reciprocal_approx_fast(rb32[0:1, :],
                                                         den[0:1, :])
                        rb16 = rbp.tile([1, tcw], F16, tag="rb16")
                        nc.vector.tensor_copy(rb16[0:1, :], rb32[0:1, :])
                        if DEBUG and hp == 0 and tc4 == 0 and j == 0:
                            nc.sync.dma_start(dbg["dbg_rb"][0:1], rb32[0:1])
                        rb16s.append(rb16)
                    return rb16s

                def normalize_apply(praw, hp, tc4, rb16s):
                    """ow = praw[0:64] * (1/den broadcast via f16 PE
                    ones-matmul; a GpSimd broadcast would force a ~7us
                    ucode library swap at every unit boundary)."""
                    t0 = tc4 * tcw
                    for j in range(2):
                        rb_ps = psO.tile([P, 512], F32, tag="po",
                                         name=f"rbps{j}")
                        nc.tensor.matmul(rb_ps[0:HD, 0:tcw],
                                         ones16[0:1, :], rb16s[j][0:1, :],
                                         start=True, stop=True)
                        nc.vector.tensor_tensor(
                            ow[hp][HD * j:HD * (j + 1), t0:t0 + tcw],
                            praw[0:HD, j, :], rb_ps[0:HD, 0:tcw],
                            op=AOP.mult)

                def normalize(praw, hp, tc4):
                    normalize_apply(praw, hp, tc4,
                                    normalize_recip(praw, hp, tc4))

                def attn_unit(hp, tc4, pend_norm, pend_oproj):
                    t0 = tc4 * tcw
                    if mask_mode == "causal":
                        s_list = [s for s in range(ns)
                                  if s * P <= t0 + tcw - 1]
                    else:
                        s_list = list(range(ns))
                    pv = psPV.tile([P, 2, tcw], F32, tag="pv",
                                   name=f"pv_{hp}_{tc4}")
                    rb16s = None
                    next_ii = 0
                    n_ii = tcw // P
                    for si, s in enumerate(s_list):
                        if pend_norm is not None:
                            if si == 1:
                                rb16s = normalize_recip(*pend_norm)
                            elif si == 3:
                                normalize_apply(*pend_norm, rb16s)
                                pend_norm = None
                        if (pend_oproj is not None and si >= 5 and si % 2
                                and next_ii < n_ii):
                            oproj_ii(pend_oproj, next_ii)
                            next_ii += 1
                        qk = psQK.tile([P, 2, tcw], F32, tag="qk")
                        for j in range(2):
                            nc.tensor.matmul(
                                qk[:, j, :],
                                kT_sb[HD * j:HD * (j + 1),
                                      s * P:(s + 1) * P],
                                qT_sb[HD * j:HD * (j + 1), hp,
                                      t0:t0 + tcw],
                                start=True, stop=True,
                                tile_position=(HD * j, 0))
                        pb = pbp.tile([P, 2, tcw], BF16, tag="pb")
                        masked = (mask_mode == "general"
                                  or (mask_mode == "causal"
                                      and (s + 1) * P > t0))
                        if masked:
                            mt = stgp.tile([P, tcw], F32, tag="mt")
                            nc.sync.dma_start(
                                mt[:],
                                maskT_d[s * P:(s + 1) * P, t0:t0 + tcw])
                            tb = stgp.tile([P, 2, tcw], F32, tag="tb")
                            nc.vector.tensor_tensor(
                                tb[:], qk[:],
                                mt[:, None, :].to_broadcast((P, 2, tcw)),
                                op=AOP.add)
                            nc.scalar.activation(pb[:], tb[:], AF.Exp,
                                                 scale=0.125)
                        else:
                            nc.scalar.activation(pb[:], qk[:], AF.Exp,
                                                 scale=0.125)
                        if DEBUG and hp == 0 and tc4 == 0 and si == 0:
                            nc.sync.dma_start(dbg["dbg_pb"][:], pb[:])
                        for j in range(2):
                            nc.tensor.matmul(
                                pv[0:HD + 1, j, :], v_sb[:, s, 0:HD + 1],
                                pb[:, j, :],
                                start=(si == 0),
                                stop=(si == len(s_list) - 1))
                    if pend_norm is not None:
                        if rb16s is None:
                            rb16s = normalize_recip(*pend_norm)
                        normalize_apply(*pend_norm, rb16s)
                    if pend_oproj is not None:
                        for ii in range(next_ii, n_ii):
                            oproj_ii(pend_oproj, ii)
                    # drain psum fast; normalize later from SBUF so the
                    # next unit's PV can reuse the pv banks immediately
                    praw = stgp.tile([HD + 1, 2, tcw], F32, tag="praw",
                                     bufs=3)
                    nc.vector.tensor_copy(praw[:], pv[0:HD + 1, :, :])
                    if DEBUG and hp == 0 and tc4 == 0:
                        nc.sync.dma_start(dbg["dbg_pv"][0:HD + 1], praw[:])
                    return praw

                pend_norm = None
                pend_oproj = None
                for tc4 in range(ntc):
                    praw = attn_unit(0, tc4, pend_norm, pend_oproj)
                    pend_norm, pend_oproj = (praw, 0, tc4), None
                    praw = attn_unit(1, tc4, pend_norm, pend_oproj)
                    pend_norm, pend_oproj = (praw, 1, tc4), tc4
                normalize(*pend_norm)
                oproj(pend_oproj, on_act=True)
                if DEBUG:
                    nc.sync.dma_start(dbg["dbg_ow"][:], ow[0][:])

    nc.finalize()
    return nc


def _get_nc(t_len, mask_mode):
    key = (t_len, mask_mode)
    if key not in _CACHE:
        _CACHE[key] = _build(t_len, mask_mode)
    return _CACHE[key]


def _host_prep(x, cos, sin, mask, wq, wk, wv, wo, q_norm_w, k_norm_w, t_len):
    f = np.float32
    bf = ml_dtypes.bfloat16
    wq, wk, wv, wo = (np.asarray(a, f) for a in (wq, wk, wv, wo))
    x = np.asarray(x, f)
    cos, sin = np.asarray(cos, f), np.asarray(sin, f)
    qw, kw = np.asarray(q_norm_w, f), np.asarray(k_norm_w, f)

    def eff(w):
        alpha = np.mean(np.abs(w), dtype=f)
        return (np.sign(w) * alpha).astype(f)

    wqe, wke, wve, woe = eff(wq), eff(wk), eff(wv), eff(wo)

    qw_sw = np.concatenate([qw[HH:], qw[:HH]])
    kw_sw = np.concatenate([kw[HH:], kw[:HH]])
    cs = {
        "cq": (cos * qw[None, :]).astype(bf),
        "sq": (sin * qw_sw[None, :]).astype(bf),
        "ck": (cos * kw[None, :]).astype(bf),
        "sk": (sin * kw_sw[None, :]).astype(bf),
    }

    m2 = np.asarray(mask, f).reshape(t_len, t_len)
    if not np.any(m2):
        mask_mode = "none"
        maskT = None
    else:
        causal = np.array_equal(
            m2, np.where(np.tril(np.ones((t_len, t_len), bool)), f(0),
                         f(-1e9)))
        mask_mode = "causal" if causal else "general"
        maskT = np.ascontiguousarray(m2.T) * f(8.0)

    in_maps = []
    for c in range(N_CORES):
        b, g = divmod(c, KVH)
        w_all = np.concatenate(
            [wqe[g * DC:(g + 1) * DC, :].T,
             wke[g * HD:(g + 1) * HD, :].T,
             wve[g * HD:(g + 1) * HD, :].T], axis=1)  # [D, 384]
        im = {
            "ident": np.eye(P, dtype=bf),
            "xT": np.ascontiguousarray(x[b].T).astype(bf),
            "wqkvT": np.ascontiguousarray(w_all).astype(bf),
            "woT": np.ascontiguousarray(woe.T[g * DC:(g + 1) * DC, :]
                                        ).astype(bf),
            **cs,
        }
        if maskT is not None:
            im["maskT"] = maskT
        in_maps.append(im)
    return in_maps, mask_mode


def kernel(x, cos, sin, mask, wq, wk, wv, wo, q_norm_w, k_norm_w,
           _trace=False, _t_len=T):
    in_maps, mask_mode = _host_prep(x, cos, sin, mask, wq, wk, wv, wo,
                                    q_norm_w, k_norm_w, _t_len)
    nc = _get_nc(_t_len, mask_mode)
    res = run_bass_kernel_spmd(nc, in_maps, core_ids=list(range(N_CORES)),
                               trace=_trace)
    out = np.zeros((B, _t_len, D), np.float32)
    for c in range(N_CORES):
        b = c // KVH
        out[b] += res.results[c]["y"]
    kernel._last = res
    return out
